# revision 1
# baseline (speedup 1.0000x reference)
"""nn_MultiHeadAttention_84954453115654 — Trainium2 Bass kernel, 8 NeuronCores.

Sharding: data-parallel over batch (2) x head-pair-parallel (4 groups of 2
heads).  Core c handles batch b = c//4 and embed-channel rows
R = [128*(c%4), 128*(c%4)+128) (= heads 2*(c%4) and 2*(c%4)+1).

Per core:
  - weight-standardize Wq/Wk/Wv row-block [128,512] and full Wo (stats over
    full rows, then slice the R columns), PE transposes for matmul layouts
  - projections (fp32r matmuls, K-chained over 4 tiles of 128); the column
    mask (x * mask) commutes through the 1x1 conv and is applied at the
    PSUM->SBUF drain (mask * psum); conv bias added after (matches ref)
  - per-head LayerNorm over DH=64 using a block-mean matmul broadcast:
    mu_b = M2^T @ x and ex2_b = M2^T @ x^2 where M2 is the per-head 1/64
    block matrix, then var/rsqrt/apply elementwise
  - scores computed TRANSPOSED: S'^T[tk,tq] = sum_d kn[d,tk] qn[d,tq] with
    1/SCALE folded into qn; key mask folded into the softmax exp as a
    per-partition bias (-80 on masked keys; exp(-80)~1e-35 which matches
    the reference's exact zeros within fp32); no row-max subtraction
    (post-LN scores are O(0.1) so exp never overflows)
  - av = V_aug^T @ exp(S') accumulated over tk tiles in PSUM; V_aug carries
    a ones column so row 64 of the result is the softmax denominator;
    the query mask is folded into the reciprocal row (this is where the
    reference's scores-row zeroing + out_proj input masking both land)
  - out_proj partial: wnoT[:, R-cols] @ (av / denom) -> [512, 2048]
Host: sums the 4 partials of each batch group and adds bo (the row-parallel
reduction of the sharding hint, performed at gather/unshard time).
"""

import os
import sys
import contextlib
import functools

for _p in ("/root/.axon_site/_ro/trn_rl_repo", "/opt/trn_rl_repo"):
    if os.path.isdir(_p) and _p not in sys.path:
        sys.path.append(_p)

import numpy as np

import concourse.bass as bass
import concourse.mybir as mybir
import concourse.tile as tile
from concourse import bass_utils, library_config

B, E, T, H = 2, 512, 2048, 8
DH = E // H            # 64
HPC = 2                # heads per core
G = 4                  # cores per batch group
NCORES = 8
NK = E // 128          # 4 contraction tiles
NJ = T // 512          # 4 time chunks
NI = T // 128          # 16 tk tiles
EPS = 1e-5
SCALE = float(E // H ** 0.5)   # 181.0
BIG = 80.0
FP = mybir.dt.float32
FR = mybir.dt.float32r
AF = mybir.ActivationFunctionType
OP = mybir.AluOpType


def _fr(ap):
    return ap.bitcast(FR)


def _split_multiwaits(nc):
    """Split multi-wait instructions (Tile's tail drain) into single-wait
    EventSemaphore chains; this container's walrus encodes only one sync
    wait per instruction."""
    import bass_rust

    n_new = 0
    for f in nc.m.functions:
        for bb in f.blocks:
            out = []
            changed = False
            for ins in bb.instructions:
                si = ins.sync_info
                if si is not None and si.on_wait is not None and len(si.on_wait) > 1:
                    waits = list(si.on_wait)
                    for w in waits[:-1]:
                        ev = bass_rust.InstEventSemaphore(
                            name=f"MWFIX-{n_new}", ins=[], outs=[]
                        )
                        n_new += 1
                        ev.engine = ins.engine
                        ev.sync_info = bass_rust.SyncInfo(on_wait=[w], on_update=[])
                        out.append(ev)
                    ins.sync_info = bass_rust.SyncInfo(
                        on_wait=[waits[-1]], on_update=list(si.on_update or [])
                    )
                    changed = True
                out.append(ins)
            if changed:
                bb.instructions = out
    return n_new


def _emit(nc, tc, dram, flags, dbg, reps=1):
    v = nc.vector
    sc = nc.scalar
    te = nc.tensor
    gp = nc.gpsimd
    sy = nc.sync

    stack = contextlib.ExitStack()
    pools = {}

    def pool(name, bufs, space="SBUF"):
        if name not in pools:
            pools[name] = stack.enter_context(
                tc.tile_pool(name=name, bufs=bufs, space=space)
            )
        return pools[name]

    consts = pool("consts", 1)
    wbuf = pool("wbuf", 1)
    mbc = pool("maskb", 1)
    xp = pool("x", 20)
    tpp = pool("tp", 1)
    sqp = pool("sq", 2)
    scr = pool("scr", 2)
    stat = pool("stat", 2)
    nbuf = pool("named", 1)
    nump = pool("numer", 4)
    rsbp = pool("rsb", 2)
    rbp = pool("rb", 6)
    outp = pool("outsb", 4)
    dramp = pool("dram_scr", 2, "DRAM")

    # ---- constant / weight loads -------------------------------------
    ident = consts.tile([128, 128], FP, tag="ident")
    sy.dma_start(ident[:], dram["ident"])
    identr = consts.tile([128, 128], FR, tag="identr")
    sy.dma_start(identr[:], dram["identr"])
    m2t = consts.tile([128, 128], FR, tag="m2")
    sy.dma_start(m2t[:], dram["m2"])
    kmt = consts.tile([128, NI], FP, tag="kmt")
    sy.dma_start(kmt[:], dram["kmt"])

    # query mask row persists (used again at softmax normalization)
    qmrow = consts.tile([1, T], FP, tag="qmrow")
    sy.dma_start(qmrow[:], dram["qm"])

    wtiles = {}
    for wname in ("wq", "wk", "wv"):
        wt = consts.tile([128, E], FP, tag=wname)
        sy.dma_start(wt[:], dram[wname])
        wtiles[wname] = wt
    wo_tiles = []
    for m in range(4):
        wt = consts.tile([128, E], FP, tag=f"wo{m}")
        sy.dma_start(wt[:], dram["wo"][128 * m : 128 * (m + 1), :])
        wo_tiles.append(wt)
    # NOTE: W tiles stay in consts (7 x 2KB); acceptable.

    bias_tiles = {}
    if flags["use_bias"]:
        for bname in ("bq", "bk", "bv"):
            bt = consts.tile([128, 1], FP, tag=bname)
            sy.dma_start(bt[:], dram[bname])
            bias_tiles[bname] = bt
    ge_tiles = {}
    if flags["use_affine"]:
        for gname in ("geq", "beq", "gek", "bek", "gev", "bev"):
            gt = consts.tile([128, 1], FP, tag=gname)
            sy.dma_start(gt[:], dram[gname])
            ge_tiles[gname] = gt

    # key-mask exp bias: (km-1)*BIG  [128, NI]
    mbias = consts.tile([128, NI], FP, tag="mbias")
    v.tensor_scalar(mbias[:], kmt[:], 1.0, BIG, op0=OP.subtract, op1=OP.mult)

    epst = consts.tile([128, 1], FP, tag="eps")
    v.memset(epst[:], EPS)

    ones64 = consts.tile([1, 64], FR, tag="ones64")
    sy.dma_start(ones64[:], dram["ones64"])

    pools.update(
        t_ident=ident, t_identr=identr, t_m2=m2t, t_kmt=kmt, t_qmrow=qmrow,
        t_wtiles=wtiles, t_wo_tiles=wo_tiles, t_bias_tiles=bias_tiles,
        t_ge_tiles=ge_tiles, t_mbias=mbias, t_eps=epst, t_ones64=ones64,
    )

    def emit_body():
        _emit_body(nc, tc, dram, flags, dbg, pools)

    for _rep in range(reps):
        emit_body()

    stack.close()


def _emit_body(nc, tc, dram, flags, dbg, pools):
    v = nc.vector
    sc = nc.scalar
    te = nc.tensor
    gp = nc.gpsimd
    sy = nc.sync
    consts = pools["consts"]
    wbuf = pools["wbuf"]
    mbc = pools["maskb"]
    xp = pools["x"]
    tpp = pools["tp"]
    sqp = pools["sq"]
    scr = pools["scr"]
    stat = pools["stat"]
    nbuf = pools["named"]
    nump = pools["numer"]
    rsbp = pools["rsb"]
    rbp = pools["rb"]
    outp = pools["outsb"]
    dramp = pools["dram_scr"]
    ident = pools["t_ident"]
    identr = pools["t_identr"]
    m2t = pools["t_m2"]
    kmt = pools["t_kmt"]
    qmrow = pools["t_qmrow"]
    wtiles = pools["t_wtiles"]
    wo_tiles = pools["t_wo_tiles"]
    bias_tiles = pools["t_bias_tiles"]
    ge_tiles = pools["t_ge_tiles"]
    mbias = pools["t_mbias"]
    epst = pools["t_eps"]
    ones64 = pools["t_ones64"]

    # ---- weight standardization + transposes -------------------------
    def w_standardize(wt, col_lo, col_n):
        s1 = stat.tile([128, 1], FP, tag="ws1")
        v.reduce_sum(s1[:], wt[:], axis=mybir.AxisListType.X)
        wsq = sqp.tile([128, E], FP, tag="wn")
        sc.activation(wsq[:], wt[:], AF.Square)
        s2 = stat.tile([128, 1], FP, tag="ws2")
        v.reduce_sum(s2[:], wsq[:], axis=mybir.AxisListType.X)
        mu = stat.tile([128, 1], FP, tag="wmu")
        gp.tensor_scalar_mul(mu[:], s1[:], 1.0 / E)
        ex2 = stat.tile([128, 1], FP, tag="wex2")
        gp.tensor_scalar_mul(ex2[:], s2[:], 1.0 / E)
        msq = stat.tile([128, 1], FP, tag="wmsq")
        gp.tensor_mul(msq[:], mu[:], mu[:])
        var = stat.tile([128, 1], FP, tag="wvar")
        gp.tensor_sub(var[:], ex2[:], msq[:])
        sd = stat.tile([128, 1], FP, tag="wsd")
        sc.activation(sd[:], var[:], AF.Sqrt, bias=epst[:])
        rsq = stat.tile([128, 1], FP, tag="wrsq")
        v.reciprocal(rsq[:], sd[:])
        wn = sqp.tile([128, col_n], FR, tag="wn")
        gp.tensor_scalar(
            wn[:],
            wt[:, col_lo : col_lo + col_n],
            mu[:],
            rsq[:],
            op0=OP.subtract,
            op1=OP.mult,
        )
        return wn

    # For q/k/v fold the per-head centering (I - M2) into the weights so the
    # projection matmul directly produces diff = p - mean_head(p):
    # (I - M2) @ (Wn @ x) = ((I - M2) Wn) @ x, and the t-column mask commutes.
    wT = {}
    with tc.tile_pool(name="ptrw", bufs=2, space="PSUM") as ptrw:
        for wname in ("wq", "wk", "wv"):
            wn = w_standardize(wtiles[wname], 0, E)
            if not flags["use_bias"]:
                pwc = ptrw.tile([128, E], FP, tag="wc")
                te.matmul(pwc[:], m2t[:], wn[:], start=True, stop=True)
                wc = sqp.tile([128, E], FR, tag="wn")
                v.tensor_sub(wc[:], wn[:], pwc[:])
            else:
                wc = wn
            wTt = wbuf.tile([128, E], FR, tag=f"{wname}T")
            for k in range(NK):
                pt = ptrw.tile([128, 128], FR, tag="wtr")
                te.transpose(pt[:], wc[:, 128 * k : 128 * (k + 1)], identr[:])
                sc.copy(wTt[:, 128 * k : 128 * (k + 1)], pt[:])
            wT[wname] = wTt

        # wo columns for this core's R were moved to the front on the host,
        # so the device always slices cols [0:128).
        woT = wbuf.tile([128, E], FR, tag="woT")
        for m in range(4):
            wn = w_standardize(wo_tiles[m], 0, 128)
            pt = ptrw.tile([128, 128], FR, tag="wtr")
            te.transpose(pt[:], wn[:], identr[:])
            sc.copy(woT[:, 128 * m : 128 * (m + 1)], pt[:])

    # value-mask broadcast early so it doesn't queue behind the x loads
    vm_mbt = mbc.tile([128, T], FP, tag="mb")
    sy.dma_start(vm_mbt[:], dram["vm"].partition_broadcast(128).squeeze(1))

    # ---- x loads ------------------------------------------------------
    # per-(ktile, tchunk) subtiles so SBUF residency stays tiny and loads
    # pipeline with the projection chunks
    xts = {}
    for tn in ("xk", "xv", "xq"):
        tiles = {}
        for j in range(NJ):
            for k in range(NK):
                xt = xp.tile([128, 512], FR, tag="x", name=f"x_{tn}_{k}_{j}")
                sy.dma_start(
                    xt[:],
                    dram[tn][128 * k : 128 * (k + 1), 512 * j : 512 * (j + 1)],
                )
                tiles[(k, j)] = xt
        xts[tn] = tiles

    # ---- projections + per-head LN ------------------------------------
    def project_ln(pjp, tn, wname, mname, bname, cscale, gname, bnameln, outname):
        # The t-column mask is only materially needed for v (value_mask):
        # masked-KEY kn columns are killed by the -BIG exp bias, and
        # masked-QUERY avn columns are zeroed at the softmax normalizer.
        use_mask = mname == "vm"
        if use_mask:
            mbt = vm_mbt
        fast = not flags["use_bias"]
        outs = []
        tp = None if fast else tpp.tile([128, T], FR, tag="tp")
        for j in range(NJ):
            js = slice(512 * j, 512 * (j + 1))
            tn_out = nbuf.tile(
                [128, 512], FR, tag=f"{outname}{j}", name=f"{outname}{j}"
            )
            outs.append(tn_out)
            pp = pjp.tile([128, 512], FP, tag="pp")
            for k in range(NK):
                te.matmul(
                    pp[:],
                    wT[wname][:, 128 * k : 128 * (k + 1)],
                    xts[tn][(k, j)][:],
                    start=(k == 0),
                    stop=(k == NK - 1),
                )
            if fast:
                # pp already holds diff = p - mean_head(p) (weights folded)
                if use_mask:
                    diff = scr.tile([128, 512], FR, tag="diff")
                    v.tensor_mul(diff[:], pp[:], mbt[:, js])
                    dsq = sqp.tile([128, 512], FR, tag="sq")
                    v.tensor_mul(dsq[:], diff[:], diff[:])
                else:
                    diff = pp
                    dsq = sqp.tile([128, 512], FR, tag="sq")
                    sc.activation(dsq[:], pp[:], AF.Square)
                pvar = pjp.tile([128, 512], FP, tag="pvar")
                te.matmul(pvar[:], m2t[:], dsq[:], start=True, stop=True)
                sd = scr.tile([128, 512], FP, tag="sd")
                sc.activation(sd[:], pvar[:], AF.Sqrt, bias=epst[:])
                rsq = scr.tile([128, 512], FP, tag="rsq")
                v.reciprocal(rsq[:], sd[:])
                v.scalar_tensor_tensor(
                    tn_out[:], diff[:], cscale, rsq[:], op0=OP.mult, op1=OP.mult
                )
            else:
                if use_mask:
                    v.tensor_mul(tp[:, js], pp[:], mbt[:, js])
                else:
                    v.tensor_copy(tp[:, js], pp[:])
                v.tensor_scalar_add(tp[:, js], tp[:, js], bias_tiles[bname][:])
                pmu = pjp.tile([128, 512], FP, tag="pvar")
                te.matmul(pmu[:], m2t[:], tp[:, js], start=True, stop=True)
                diff = scr.tile([128, 512], FR, tag="diff")
                v.tensor_sub(diff[:], tp[:, js], pmu[:])
                dsq = sqp.tile([128, 512], FR, tag="sq")
                v.tensor_mul(dsq[:], diff[:], diff[:])
                pvar = pjp.tile([128, 512], FP, tag="pvar")
                te.matmul(pvar[:], m2t[:], dsq[:], start=True, stop=True)
                sd = scr.tile([128, 512], FP, tag="sd")
                sc.activation(sd[:], pvar[:], AF.Sqrt, bias=epst[:])
                rsq = scr.tile([128, 512], FP, tag="rsq")
                v.reciprocal(rsq[:], sd[:])
                v.scalar_tensor_tensor(
                    tn_out[:], diff[:], cscale, rsq[:], op0=OP.mult, op1=OP.mult
                )
            if flags["use_affine"]:
                v.tensor_scalar(
                    tn_out[:],
                    tn_out[:],
                    ge_tiles[gname][:],
                    ge_tiles[bnameln][:],
                    op0=OP.mult,
                    op1=OP.add,
                )
        return outs

    with tc.tile_pool(name="pj", bufs=3, space="PSUM") as pjp:
        kn = project_ln(pjp, "xk", "wk", "km", "bk", 1.0, "gek", "bek", "kn")
        vn = project_ln(pjp, "xv", "wv", "vm", "bv", 1.0, "gev", "bev", "vn")

        # ---- v transpose -> vaugT [128, 32*65] ------------------------
        # (before q's LN so the transposes overlap it off the critical path)
        vaug = nbuf.tile([128, NI * HPC * 65], FR, tag="vaug")
        vaug3 = vaug[:].rearrange("p (n c) -> p n c", c=65)
        # whole-tile memset to 1.0; the transposed v blocks overwrite cols
        # 0..63 of every 65-block, leaving col 64 as the denominator ones.
        v.memset(vaug[:].bitcast(FP), 1.0)
        with tc.tile_pool(name="ptrv", bufs=2, space="PSUM") as ptrv:
            for i in range(NI):
                pt = ptrv.tile([128, 128], FR, tag="vtr")
                te.transpose(
                    pt[:], vn[i // 4][:, 128 * (i % 4) : 128 * (i % 4 + 1)], identr[:]
                )
                for h in range(HPC):
                    eng = v if (i + h) % 2 == 0 else sc
                    if eng is v:
                        v.tensor_copy(
                            vaug3[:, HPC * i + h, 0:64], pt[:, 64 * h : 64 * (h + 1)]
                        )
                    else:
                        sc.copy(
                            vaug3[:, HPC * i + h, 0:64], pt[:, 64 * h : 64 * (h + 1)]
                        )
        qn = project_ln(pjp, "xq", "wq", "qm", "bq", 1.0 / SCALE, "geq", "beq", "qn")

    # ---- attention -----------------------------------------------------
    # Processed in two tq-halves (jj); within a half both heads interleave
    # so score matmuls (K=64, partition bases 0/64) can run concurrently on
    # PE row-groups and the ACT exp stream stays saturated.
    avn = [
        nbuf.tile([128, 512], FR, tag=f"avn{j}", name=f"avn{j}") for j in range(NJ)
    ]
    late_avsb = {}
    with (
        tc.tile_pool(name="ps", bufs=2, space="PSUM") as pss,
        tc.tile_pool(name="pav", bufs=4, space="PSUM") as pav,
    ):
        for jj in range(2):
            av_tiles = {
                (h, jp): pav.tile([65, 512], FP, tag="av", name=f"av{jj}_{h}_{jp}")
                for h in range(HPC)
                for jp in range(2)
            }
            for i in range(NI):
                isl = slice(128 * i, 128 * (i + 1))
                for h in range(HPC):
                    hs = slice(64 * h, 64 * (h + 1))
                    ps = pss.tile([128, 1024], FP, tag="ps", name=f"ps{jj}_{i}_{h}")
                    kslc = kn[i // 4][hs, 128 * (i % 4) : 128 * (i % 4 + 1)]
                    for jp in range(2):
                        j = 2 * jj + jp
                        te.matmul(
                            ps[:, 512 * jp : 512 * (jp + 1)],
                            kslc,
                            qn[j][hs, :],
                            start=True,
                            stop=True,
                        )
                    nt = nump.tile([128, 1024], FR, tag="numer")
                    sc.activation(nt[:], ps[:], AF.Exp, bias=mbias[:, i : i + 1])
                    for jp in range(2):
                        te.matmul(
                            av_tiles[(h, jp)][:],
                            vaug3[:, HPC * i + h, :],
                            nt[:, 512 * jp : 512 * (jp + 1)],
                            start=(i == 0),
                            stop=(i == NI - 1),
                        )
            for h in range(HPC):
                hs = slice(64 * h, 64 * (h + 1))
                for jp in range(2):
                    j = 2 * jj + jp
                    js = slice(512 * j, 512 * (j + 1))
                    avt = av_tiles[(h, jp)]
                    # drain PSUM immediately so the next jj's chains can
                    # allocate their banks without waiting
                    avsb = rbp.tile(
                        [65, 512], FP, tag="avsb", name=f"avsb{jj}_{h}_{jp}"
                    )
                    v.tensor_copy(avsb[:], avt[:])
                    if jj == 0:
                        # overlapped with the jj=1 attention pass: DRAM-bounce
                        # broadcast of qm/denominator
                        rsb = rsbp.tile([1, 512], FP, tag="rsb")
                        v.reciprocal(rsb[:], avsb[64:65, :])
                        v.tensor_mul(rsb[:], rsb[:], qmrow[:, js])
                        dscr = dramp.tile([1, 512], FP, tag="dscr")
                        sy.dma_start(dscr[:], rsb[:])
                        rb = rbp.tile([64, 512], FP, tag="rb")
                        sy.dma_start(rb[:], dscr[:].partition_broadcast(64).squeeze(1))
                        v.tensor_mul(avn[j][hs, :], avsb[0:64, :], rb[:])
                    else:
                        late_avsb[(h, jp)] = avsb

    # ---- out_proj partial + store -------------------------------------
    # j=0,1 (ready from the first attention pass) go out immediately; the
    # second pass's softmax normalization runs concurrently using a matmul
    # broadcast (PSUM banks are free now), then j=2,3 follow.
    with tc.tile_pool(name="pout", bufs=4, space="PSUM") as poutp:

        def outproj(j):
            js = slice(512 * j, 512 * (j + 1))
            for m in range(4):
                po = poutp.tile([128, 512], FP, tag="pout", name=f"po{j}_{m}")
                te.matmul(
                    po[:],
                    woT[:, 128 * m : 128 * (m + 1)],
                    avn[j][:],
                    start=True,
                    stop=True,
                )
                ot = outp.tile([128, 512], FP, tag="outsb", name=f"ot{j}_{m}")
                if (j * 4 + m) % 2 == 0:
                    v.tensor_copy(ot[:], po[:])
                else:
                    sc.copy(ot[:], po[:])
                sy.dma_start(dram["out"][128 * m : 128 * (m + 1), js], ot[:])

        outproj(0)
        outproj(1)
        for h in range(HPC):
            hs = slice(64 * h, 64 * (h + 1))
            for jp in range(2):
                j = 2 + jp
                js = slice(512 * j, 512 * (j + 1))
                avsb = late_avsb[(h, jp)]
                rsb = rsbp.tile([1, 512], FP, tag="rsb", name=f"rsb2{h}_{jp}")
                v.reciprocal(rsb[:], avsb[64:65, :])
                rsbr = rsbp.tile([1, 512], FR, tag="rsbr", name=f"rsbr{h}_{jp}")
                v.tensor_mul(rsbr[:], rsb[:], qmrow[:, js])
                rbp_ps = poutp.tile([64, 512], FP, tag="rbp", name=f"rbp{h}_{jp}")
                te.matmul(rbp_ps[:], ones64[:], rsbr[:], start=True, stop=True)
                v.tensor_mul(avn[j][hs, :], avsb[0:64, :], rbp_ps[:])
        outproj(2)
        outproj(3)




@functools.lru_cache(maxsize=4)
def _build(use_bias, use_affine, debug_names, reps=1):
    nc = bass.Bass(
        "TRN2", target_bir_lowering=False, debug=False, num_devices=NCORES
    )
    dram = {}
    for tn in ("xq", "xk", "xv"):
        dram[tn] = nc.dram_tensor(tn, [E, T], FR, kind="ExternalInput").ap()
    for wn in ("wq", "wk", "wv"):
        dram[wn] = nc.dram_tensor(wn, [128, E], FP, kind="ExternalInput").ap()
    dram["wo"] = nc.dram_tensor("wo", [E, E], FP, kind="ExternalInput").ap()
    for mn in ("qm", "km", "vm"):
        dram[mn] = nc.dram_tensor(mn, [1, T], FP, kind="ExternalInput").ap()
    dram["kmt"] = nc.dram_tensor("kmt", [128, NI], FP, kind="ExternalInput").ap()
    dram["ident"] = nc.dram_tensor("ident", [128, 128], FP, kind="ExternalInput").ap()
    dram["m2"] = nc.dram_tensor("m2", [128, 128], FR, kind="ExternalInput").ap()
    dram["identr"] = nc.dram_tensor("identr", [128, 128], FR, kind="ExternalInput").ap()
    dram["ones64"] = nc.dram_tensor("ones64", [1, 64], FR, kind="ExternalInput").ap()
    if use_bias:
        for bn in ("bq", "bk", "bv"):
            dram[bn] = nc.dram_tensor(bn, [128, 1], FP, kind="ExternalInput").ap()
    if use_affine:
        for gn in ("geq", "beq", "gek", "bek", "gev", "bev"):
            dram[gn] = nc.dram_tensor(gn, [128, 1], FP, kind="ExternalInput").ap()
    dram["out"] = nc.dram_tensor("out", [E, T], FP, kind="ExternalOutput").ap()
    dbg = frozenset(debug_names.split(",")) - {""} if debug_names else frozenset()
    for dname in dbg:
        dram["dbg_" + dname] = nc.dram_tensor(
            "dbg_" + dname, [128, T], FP, kind="ExternalOutput"
        ).ap()

    flags = {"use_bias": use_bias, "use_affine": use_affine}
    with tile.TileContext(nc) as tc:
        _emit(nc, tc, dram, flags, dbg, reps=reps)
    _split_multiwaits(nc)
    return nc


@functools.lru_cache(maxsize=1)
def _m2_const():
    m2 = np.zeros((128, 128), np.float32)
    m2[:64, :64] = 1.0 / DH
    m2[64:, 64:] = 1.0 / DH
    return m2


def _prep_core_inputs(c, a):
    b, hp = divmod(c, G)
    rs = 128 * hp
    wo_perm = np.concatenate(
        [a["Wo"][:, rs : rs + 128], np.delete(a["Wo"], np.s_[rs : rs + 128], axis=1)],
        axis=1,
    )
    d = {
        "xq": a["q"][b],
        "xk": a["k"][b],
        "xv": a["v"][b],
        "wq": a["Wq"][rs : rs + 128],
        "wk": a["Wk"][rs : rs + 128],
        "wv": a["Wv"][rs : rs + 128],
        "wo": wo_perm,
        "qm": a["query_mask"][b].astype(np.float32)[None, :],
        "km": a["key_mask"][b].astype(np.float32)[None, :],
        "vm": a["value_mask"][b].astype(np.float32)[None, :],
        "kmt": a["key_mask"][b].astype(np.float32).reshape(NI, 128).T,
        "ident": np.eye(128, dtype=np.float32),
        "identr": np.eye(128, dtype=np.float32),
        "ones64": np.ones((1, 64), np.float32),
        "m2": _m2_const(),
    }
    return d


_last_results = None


def kernel(**inputs):
    global _last_results
    a = {k: np.asarray(val) for k, val in inputs.items()}
    use_bias = bool(any(np.any(a[bn] != 0) for bn in ("bq", "bk", "bv")))
    use_affine = bool(
        any(np.any(a[gn] != 1) for gn in ("ln_gq", "ln_gk", "ln_gv"))
        or any(np.any(a[bn] != 0) for bn in ("ln_bq", "ln_bk", "ln_bv"))
    )
    debug_names = os.environ.get("KDEBUG", "")

    nc = _build(use_bias, use_affine, debug_names)

    in_maps = []
    for c in range(NCORES):
        d = _prep_core_inputs(c, a)
        b, hp = divmod(c, G)
        rs = 128 * hp
        if use_bias:
            d["bq"] = a["bq"][rs : rs + 128][:, None]
            d["bk"] = a["bk"][rs : rs + 128][:, None]
            d["bv"] = a["bv"][rs : rs + 128][:, None]
        if use_affine:
            d["geq"] = np.tile(a["ln_gq"], HPC)[:, None]
            d["beq"] = (np.tile(a["ln_bq"], HPC) / SCALE)[:, None]
            d["gek"] = np.tile(a["ln_gk"], HPC)[:, None]
            d["bek"] = np.tile(a["ln_bk"], HPC)[:, None]
            d["gev"] = np.tile(a["ln_gv"], HPC)[:, None]
            d["bev"] = np.tile(a["ln_bv"], HPC)[:, None]
        d = {
            k: np.ascontiguousarray(val, dtype=np.float32) for k, val in d.items()
        }
        in_maps.append(d)

    res = bass_utils.run_bass_kernel_spmd(
        nc,
        in_maps,
        core_ids=list(range(NCORES)),
        trace=os.environ.get("KTRACE", "0") == "1",
    )
    _last_results = res

    out = np.zeros((B, E, T), np.float32)
    bo = a["bo"].astype(np.float32)
    for b in range(B):
        acc = res.results[G * b]["out"].astype(np.float32).copy()
        for c in range(G * b + 1, G * b + G):
            acc += res.results[c]["out"]
        out[b] = acc + bo[:, None]
    return out



# revision 45
# speedup vs baseline: 1.7705x; 1.7705x over previous
"""nn_MultiHeadAttention_84954453115654 — Trainium2 Bass kernel, 8 NeuronCores.

Sharding: data-parallel over batch (2) x head-pair-parallel (4 groups of 2
heads).  Core c handles batch b = c//4 and embed rows [128*(c%4), +128)
(= heads 2*(c%4), 2*(c%4)+1).  Host sums the 4 out_proj partials per batch
and adds bo (row-parallel all-reduce done at gather time).

Key idea: the masks are inputs, so the host COMPACTS the time axis before
launch.  Only unmasked query columns (Tq' ~ 1024 of 2048) and unmasked key
columns (Tk' ~ 1024) are shipped/computed; value_mask is pre-applied to the
compacted xv.  Masked query columns of the output are exactly bo (reference:
scores row zeroed -> av col 0 -> conv of masked col -> bias), so the host
scatters computed columns back and fills the rest with bo.  This halves DMA
and PE work and quarters the softmax-exp ACT work vs. the dense formulation.

Weight prep happens on host (weights are inputs): weight-standardize,
fold the per-head centering (I - M2) into Wq/Wk/Wv so the projection
matmul directly yields diff = p - mean_head(p), transpose into lhsT
layout, cast bf16.

Per core device program:
  - q/k/v projections (bf16 matmuls, K-chained over 4 tiles of 128) with
    fused per-head LayerNorm: var via block-mean matmul of diff^2, then
    out = (diff * cscale) / sqrt(var + eps) in one DVE op (1/SCALE folded
    into q's cscale)
  - scores transposed per tk-tile i: S^T[tk, tq] = kn_i^T @ qn, one
    [128, Tq] PSUM tile per (i, head); softmax exp on ACT with a
    per-partition bias (-80 on compact-pad keys, else 0); no row-max
    subtraction (post-LN scores are O(0.1))
  - av[65, 512] += vaugT_i @ exp accumulated over i in PSUM; vaug carries a
    ones column so row 64 is the softmax denominator
  - normalize: denominator row -> SBUF, PE-broadcast over 64 partitions,
    avn = av / bcast (no query-mask term: all compacted queries are live)
  - out_proj partial: woT[:, m] @ avn -> [512, Tq] stored bf16
"""

import os
import sys
import contextlib
import functools

for _p in ("/root/.axon_site/_ro/trn_rl_repo", "/opt/trn_rl_repo"):
    if os.path.isdir(_p) and _p not in sys.path:
        sys.path.append(_p)

import numpy as np
import ml_dtypes

import concourse.bass as bass
import concourse.mybir as mybir
import concourse.tile as tile
from concourse import bass_utils

B, E, T, H = 2, 512, 2048, 8
DH = E // H            # 64
HPC = 2                # heads per core
G = 4                  # cores per batch group
NCORES = 8
NK = E // 128          # 4 contraction tiles for the projections
EPS = 1e-5
SCALE = float(E // H ** 0.5)   # 181.0
BIG = 80.0
FP = mybir.dt.float32
FR = mybir.dt.float32r
BF = mybir.dt.bfloat16
AF = mybir.ActivationFunctionType
OP = mybir.AluOpType
BF_NP = np.dtype(ml_dtypes.bfloat16)


def _split_multiwaits(nc):
    """Split multi-wait instructions (Tile's tail drain) into single-wait
    EventSemaphore chains; this container's walrus encodes only one sync
    wait per instruction."""
    import bass_rust

    n_new = 0
    for f in nc.m.functions:
        for bb in f.blocks:
            out = []
            changed = False
            for ins in bb.instructions:
                si = ins.sync_info
                if si is not None and si.on_wait is not None and len(si.on_wait) > 1:
                    waits = list(si.on_wait)
                    for w in waits[:-1]:
                        ev = bass_rust.InstEventSemaphore(
                            name=f"MWFIX-{n_new}", ins=[], outs=[]
                        )
                        n_new += 1
                        ev.engine = ins.engine
                        ev.sync_info = bass_rust.SyncInfo(on_wait=[w], on_update=[])
                        out.append(ev)
                    ins.sync_info = bass_rust.SyncInfo(
                        on_wait=[waits[-1]], on_update=list(si.on_update or [])
                    )
                    changed = True
                out.append(ins)
            if changed:
                bb.instructions = out
    return n_new


def _chunks(total, step=512):
    """[(off, width), ...] covering `total` in steps of `step`."""
    out = []
    off = 0
    while off < total:
        out.append((off, min(step, total - off)))
        off += step
    return out


def _emit(nc, tc, dram, flags, dbg, Tq, Tk):
    v = nc.vector
    sc = nc.scalar
    te = nc.tensor
    gp = nc.gpsimd
    sy = nc.sync

    NIK = Tk // 128          # tk tiles
    JQ = _chunks(Tq)         # tq chunks (512-wide, last may be ragged)
    JK = _chunks(Tk)
    JT = _chunks(Tq, 1024)   # scores/exp tiles (<= 2 PSUM banks each)
    use_bias = flags["use_bias"]
    use_affine = flags["use_affine"]

    stack = contextlib.ExitStack()

    consts = stack.enter_context(tc.tile_pool(name="consts", bufs=1))
    xp = stack.enter_context(tc.tile_pool(name="x", bufs=12))
    nbuf = stack.enter_context(tc.tile_pool(name="named", bufs=1))
    ntp = stack.enter_context(
        tc.tile_pool(name="nt", bufs=NIK * HPC * len(_chunks(Tq, 1024)) + 2)
    )
    sdp = stack.enter_context(tc.tile_pool(name="sd", bufs=3))
    dnp = stack.enter_context(tc.tile_pool(name="dn", bufs=4))
    outp = stack.enter_context(tc.tile_pool(name="outsb", bufs=4))

    # ---- constant / weight loads (DMA queue order matters) -------------
    wT = {}
    for wname in ("wq", "wk", "wv"):
        wt = consts.tile([128, E], BF, tag=wname, name=wname)
        sy.dma_start(wt[:], dram[wname])
        wT[wname] = wt
    m2b = consts.tile([128, 128], BF, tag="m2b")
    sy.dma_start(m2b[:], dram["m2b"])
    identr = consts.tile([128, 128], FR, tag="identr")
    sy.dma_start(identr[:], dram["identr"])
    ones64 = consts.tile([1, DH], FR, tag="ones64")
    sy.dma_start(ones64[:], dram["ones64"])
    mbias = consts.tile([128, NIK], FP, tag="mbias")
    sy.dma_start(mbias[:], dram["mbias"])
    epst = consts.tile([128, 1], FP, tag="eps")
    v.memset(epst[:], EPS)

    bias_tiles = {}
    if use_bias:
        for bn in ("bcq", "bck", "bcv"):
            bt = consts.tile([128, 1], FP, tag=bn, name=bn)
            sy.dma_start(bt[:], dram[bn])
            bias_tiles[bn] = bt
    ge_tiles = {}
    if use_affine:
        for gn in ("geq", "beq", "gek", "bek", "gev", "bev"):
            gt = consts.tile([128, 1], FP, tag=gn, name=gn)
            sy.dma_start(gt[:], dram[gn])
            ge_tiles[gn] = gt

    # ---- x loads: xq first (first exp needs all of qn), then xk, xv ----
    xts = {}
    for tn, Tx in (("xq", Tq), ("xk", Tk), ("xv", Tk)):
        tiles = {}
        for j, (joff, jw) in enumerate(_chunks(Tx)):
            for k in range(NK):
                xt = xp.tile([128, jw], BF, tag=f"x{jw}", name=f"x_{tn}_{k}_{j}")
                sy.dma_start(
                    xt[:], dram[tn][128 * k : 128 * (k + 1), joff : joff + jw]
                )
                tiles[(k, j)] = xt
        xts[tn] = tiles
    woT = consts.tile([128, E], BF, tag="woT")
    sy.dma_start(woT[:], dram["woT"])

    # named projection outputs
    qn = nbuf.tile([128, Tq], BF, tag="qn", name="qn")
    kn = nbuf.tile([128, Tk], BF, tag="kn", name="kn")
    vn = nbuf.tile([128, Tk], FR, tag="vn", name="vn")  # fp32r: feeds PE transpose
    tn_tiles = {"xq": qn, "xk": kn, "xv": vn}
    # vaugT: per (i, h) a [128, 65] block: cols 0..63 = v^T, col 64 = ones
    vaug = nbuf.tile([128, NIK * HPC * 65], BF, tag="vaug", name="vaug")
    vaug3 = vaug[:].rearrange("p (n c) -> p n c", c=65)
    gp.memset(vaug[:], 1.0)
    avn = nbuf.tile([128, Tq], BF, tag="avn", name="avn")

    # PSUM pool stack: ps at the bottom (lives through normalize/out_proj,
    # whose rbp/po tiles ride the same tag ring), pj on top (released once
    # all projections are emitted), then pav groups.
    ps_pool = tc.alloc_tile_pool(name="ps", bufs=2, space="PSUM")
    pj = tc.alloc_tile_pool(name="pj", bufs=2, space="PSUM")

    def project_chunk(tn, wname, j, joff, jw, cscale, bn, gn, bln, dsq_eng):
        """Project x chunk j and apply per-head LN; writes tn_tiles[tn]."""
        out = tn_tiles[tn]
        pp = pj.tile([128, 512], FP, tag="pp", name=f"pp_{tn}_{j}")
        for k in range(NK):
            te.matmul(
                pp[:, :jw],
                wT[wname][:, 128 * k : 128 * (k + 1)],
                xts[tn][(k, j)][:],
                start=(k == 0),
                stop=(k == NK - 1),
            )
        if use_bias or dsq_eng is not sc:
            # SBUF copy of diff: PSUM-reading two-input ops are illegal, so
            # non-ACT squares (and the bias add) go through SBUF
            u = sdp.tile([128, 512], FP, tag="u", name=f"u_{tn}_{j}")
            if use_bias:
                v.tensor_scalar_add(u[:, :jw], pp[:, :jw], bias_tiles[bn][:])
            else:
                v.tensor_scalar_add(u[:, :jw], pp[:, :jw], 0.0)
            diff = u
        else:
            diff = pp
        dsq = sdp.tile([128, 512], BF, tag="dsq", name=f"dsq_{tn}_{j}")
        if dsq_eng is sc:
            sc.activation(dsq[:, :jw], diff[:, :jw], AF.Square)
        else:
            dsq_eng.tensor_mul(dsq[:, :jw], diff[:, :jw], diff[:, :jw])
        pvar = pj.tile([128, 512], FP, tag="pvar", bufs=1, name=f"pvar_{tn}_{j}")
        te.matmul(pvar[:, :jw], m2b[:], dsq[:, :jw], start=True, stop=True)
        sd = sdp.tile([128, 512], FP, tag="sd", name=f"sd_{tn}_{j}")
        sc.activation(sd[:, :jw], pvar[:, :jw], AF.Sqrt, bias=epst[:])
        rsq = sdp.tile([128, 512], FP, tag="rsq", name=f"rsq_{tn}_{j}")
        v.reciprocal(rsq[:, :jw], sd[:, :jw])
        v.scalar_tensor_tensor(
            out[:, joff : joff + jw],
            diff[:, :jw],
            cscale,
            rsq[:, :jw],
            op0=OP.mult,
            op1=OP.mult,
        )
        if use_affine:
            v.tensor_scalar(
                out[:, joff : joff + jw],
                out[:, joff : joff + jw],
                ge_tiles[gn][:],
                ge_tiles[bln][:],
                op0=OP.mult,
                op1=OP.add,
            )

    def vchunk(j, joff, jw):
        """v projection chunk + transposes of its tk tiles into vaug."""
        project_chunk("xv", "wv", j, joff, jw, 1.0, "bcv", "gev", "bev", gp)
        for ii in range(jw // 128):
            i = joff // 128 + ii
            pt = pj.tile([128, 128], FR, tag="ptr", bufs=1, name=f"ptr{i}")
            te.transpose(pt[:], vn[:, 128 * i : 128 * (i + 1)], identr[:])
            # both heads in one strided copy: [128, (h), 64] -> vaug blocks
            src = pt[:].bitcast(FP).rearrange("p (h c) -> p h c", c=DH)
            dst = vaug3[:, HPC * i : HPC * i + HPC, 0:DH]
            v.tensor_copy(dst, src)

    # ---- emission: q proj, k proj, then attention with v interleaved ---
    for j, (joff, jw) in enumerate(JQ):
        project_chunk("xq", "wq", j, joff, jw, 1.0 / SCALE, "bcq", "geq", "beq", sc)
    project_chunk("xk", "wk", 0, JK[0][0], JK[0][1], 1.0, "bck", "gek", "bek", v)

    # ---- attention ----------------------------------------------------
    # per (i, h): one [128, Tq] scores tile -> exp -> nt (SBUF); av chains
    # for the first JQ group stream behind the exps once pj's banks free up
    groups = [list(enumerate(JQ))[g0 : g0 + 2] for g0 in range(0, len(JQ), 2)]

    def make_av_tiles(pav, grp):
        return {
            (h, j): pav.tile([DH + 1, jw], FP, tag=f"av{h}_{j}", name=f"av{h}_{j}")
            for h in range(HPC)
            for j, (joff, jw) in grp
        }

    def av_step(av_tiles, grp, i):
        for h in range(HPC):
            for j, (joff, jw) in grp:
                jt, off = joff // 1024, joff % 1024
                te.matmul(
                    av_tiles[(h, j)][:],
                    vaug3[:, HPC * i + h, :],
                    nts[(i, h, jt)][:, off : off + jw],
                    start=(i == 0),
                    stop=(i == NIK - 1),
                )

    nts = {}
    vi = 0  # next v chunk to emit
    ki = 1  # next k chunk to emit
    pav = None
    for i in range(NIK):
        # interleave k/v projection chunks so PE work lands between scores
        if i >= 2 and vi < len(JK):
            vchunk(vi, *JK[vi])
            vi += 1
        if ki < len(JK) and i + 1 >= 4 * ki:
            project_chunk("xk", "wk", ki, *JK[ki], 1.0, "bck", "gek", "bek", v)
            ki += 1
        for h in range(HPC):
            hs = slice(DH * h, DH * (h + 1))
            for jt, (toff, tw) in enumerate(JT):
                ps = ps_pool.tile([128, 1024], FP, tag="ps", name=f"ps{i}_{h}_{jt}")
                for soff, sw in _chunks(tw):
                    te.matmul(
                        ps[:, soff : soff + sw],
                        kn[hs, 128 * i : 128 * (i + 1)],
                        qn[hs, toff + soff : toff + soff + sw],
                        start=True,
                        stop=True,
                    )
                nt = ntp.tile([128, 1024], BF, tag="nt", name=f"nt{i}_{h}_{jt}")
                sc.activation(nt[:, :tw], ps[:, :tw], AF.Exp, bias=mbias[:, i : i + 1])
                nts[(i, h, jt)] = nt
        if pav is None and vi == len(JK) and ki == len(JK):
            pj.release()
            pav = tc.alloc_tile_pool(name="pav0", bufs=1, space="PSUM")
            av_tiles = make_av_tiles(pav, groups[0])
            for ii in range(i + 1):
                av_step(av_tiles, groups[0], ii)
        elif pav is not None:
            av_step(av_tiles, groups[0], i)
    if pav is None:
        while vi < len(JK):
            vchunk(vi, *JK[vi])
            vi += 1
        pj.release()
        pav = tc.alloc_tile_pool(name="pav0", bufs=1, space="PSUM")
        av_tiles = make_av_tiles(pav, groups[0])
        for ii in range(NIK):
            av_step(av_tiles, groups[0], ii)

    # ---- normalize + out_proj + store ---------------------------------
    # rbp/po PSUM tiles ride the ps pool's tag ring (banks free post-exp)
    def norm_and_out(grp, av_tiles):
        for j, (joff, jw) in grp:
            for h in range(HPC):
                hs = slice(DH * h, DH * (h + 1))
                avt = av_tiles[(h, j)]
                avsb = dnp.tile([DH, 512], FP, tag="avsb", name=f"avsb{h}_{j}")
                v.tensor_copy(avsb[:, :jw], avt[0:DH, :])
                dnm = dnp.tile([1, 512], FP, tag="dnm", name=f"dnm{h}_{j}")
                sc.copy(dnm[:, :jw], avt[DH : DH + 1, :])
                rdn = dnp.tile([1, 512], FR, tag="rdn", name=f"rdn{h}_{j}")
                with nc.allow_low_precision(reason="fp32r broadcast of recip row"):
                    v.reciprocal(rdn[:, :jw], dnm[:, :jw])
                rbp = ps_pool.tile([128, 1024], FP, tag="ps", name=f"rbp{h}_{j}")
                te.matmul(
                    rbp[0:DH, :jw],
                    ones64[:],
                    rdn[:, :jw],
                    start=True,
                    stop=True,
                )
                v.tensor_mul(
                    avn[hs, joff : joff + jw],
                    avsb[:, :jw],
                    rbp[0:DH, :jw],
                )
        for j, (joff, jw) in grp:
            for m in range(4):
                po = ps_pool.tile([128, 1024], FP, tag="ps", name=f"po{j}_{m}")
                te.matmul(
                    po[:, :jw],
                    woT[:, 128 * m : 128 * (m + 1)],
                    avn[:, joff : joff + jw],
                    start=True,
                    stop=True,
                )
                ot = outp.tile([128, 512], BF, tag="ot", name=f"ot{j}_{m}")
                if m % 2 == 0:
                    v.tensor_copy(ot[:, :jw], po[:, :jw])
                else:
                    sc.copy(ot[:, :jw], po[:, :jw])
                sy.dma_start(
                    dram["out"][128 * m : 128 * (m + 1), joff : joff + jw],
                    ot[:, :jw],
                )

    norm_and_out(groups[0], av_tiles)
    pav.release()
    for gi, grp in enumerate(groups[1:], 1):
        pav = tc.alloc_tile_pool(name=f"pav{gi}", bufs=1, space="PSUM")
        av_tiles = make_av_tiles(pav, grp)
        for ii in range(NIK):
            av_step(av_tiles, grp, ii)
        norm_and_out(grp, av_tiles)
        pav.release()
    ps_pool.release()

    for dname in dbg:
        src = {"qn": qn, "kn": kn, "vn": vn, "avn": avn}[dname]
        sy.dma_start(dram["dbg_" + dname][:, : src.shape[1]], src[:])

    stack.close()


_last_dims = (1024, 1024)


def _build(use_bias, use_affine, debug_names="", Tq=None, Tk=None):
    if Tq is None or Tk is None:
        Tq, Tk = _last_dims
    return _build_impl(use_bias, use_affine, debug_names, Tq, Tk)


@functools.lru_cache(maxsize=4)
def _build_impl(use_bias, use_affine, debug_names, Tq, Tk):
    nc = bass.Bass(
        "TRN2", target_bir_lowering=False, debug=False, num_devices=NCORES
    )
    NIK = Tk // 128
    dram = {}
    dram["xq"] = nc.dram_tensor("xq", [E, Tq], BF, kind="ExternalInput").ap()
    dram["xk"] = nc.dram_tensor("xk", [E, Tk], BF, kind="ExternalInput").ap()
    dram["xv"] = nc.dram_tensor("xv", [E, Tk], BF, kind="ExternalInput").ap()
    for wn in ("wq", "wk", "wv", "woT"):
        dram[wn] = nc.dram_tensor(wn, [128, E], BF, kind="ExternalInput").ap()
    dram["m2b"] = nc.dram_tensor("m2b", [128, 128], BF, kind="ExternalInput").ap()
    dram["identr"] = nc.dram_tensor("identr", [128, 128], FR, kind="ExternalInput").ap()
    dram["ones64"] = nc.dram_tensor("ones64", [1, DH], FR, kind="ExternalInput").ap()
    dram["mbias"] = nc.dram_tensor("mbias", [128, NIK], FP, kind="ExternalInput").ap()
    if use_bias:
        for bn in ("bcq", "bck", "bcv"):
            dram[bn] = nc.dram_tensor(bn, [128, 1], FP, kind="ExternalInput").ap()
    if use_affine:
        for gn in ("geq", "beq", "gek", "bek", "gev", "bev"):
            dram[gn] = nc.dram_tensor(gn, [128, 1], FP, kind="ExternalInput").ap()
    dram["out"] = nc.dram_tensor("out", [E, Tq], BF, kind="ExternalOutput").ap()
    dbg = frozenset(debug_names.split(",")) - {""} if debug_names else frozenset()
    for dname in dbg:
        w = Tq if dname in ("qn", "avn") else Tk
        dram["dbg_" + dname] = nc.dram_tensor(
            "dbg_" + dname, [128, w], BF, kind="ExternalOutput"
        ).ap()

    flags = {"use_bias": use_bias, "use_affine": use_affine}
    with tile.TileContext(nc) as tc:
        _emit(nc, tc, dram, flags, dbg, Tq, Tk)
    _split_multiwaits(nc)
    return nc


def _pad_up(n, m):
    return max(m, ((n + m - 1) // m) * m)


@functools.lru_cache(maxsize=1)
def _m2_const():
    m2 = np.zeros((128, 128), np.float32)
    m2[:DH, :DH] = 1.0 / DH
    m2[DH:, DH:] = 1.0 / DH
    return m2


def _std(w):
    mu = w.mean(axis=1, keepdims=True)
    var = w.var(axis=1, keepdims=True)
    return (w - mu) / np.sqrt(var + EPS)


_last_results = None


def kernel(**inputs):
    global _last_results
    a = {k: np.asarray(val) for k, val in inputs.items()}
    use_bias = bool(any(np.any(a[bn] != 0) for bn in ("bq", "bk", "bv")))
    use_affine = bool(
        any(np.any(a[gn] != 1) for gn in ("ln_gq", "ln_gk", "ln_gv"))
        or any(np.any(a[bn] != 0) for bn in ("ln_bq", "ln_bk", "ln_bv"))
    )
    debug_names = os.environ.get("KDEBUG", "")

    qm = a["query_mask"].astype(bool)
    km = a["key_mask"].astype(bool)
    kept_q = [np.flatnonzero(qm[b]) for b in range(B)]
    kept_k = [np.flatnonzero(km[b]) for b in range(B)]
    nq = [len(ix) for ix in kept_q]
    nk = [len(ix) for ix in kept_k]
    Tq = _pad_up(max(nq), 128)
    Tk = _pad_up(max(nk), 128)
    NIK = Tk // 128

    global _last_dims
    _last_dims = (Tq, Tk)
    nc = _build(use_bias, use_affine, debug_names, Tq, Tk)

    # host weight prep (fp32), shared across cores of the same head group
    m2 = _m2_const()
    i128 = np.eye(128, dtype=np.float32)
    wsn = {wn: _std(a[wn].astype(np.float32)) for wn in ("Wq", "Wk", "Wv", "Wo")}

    in_maps = []
    for c in range(NCORES):
        b, hp = divmod(c, G)
        rs = 128 * hp
        d = {}
        # compact + pad x; value_mask pre-applied to xv
        xq = np.zeros((E, Tq), np.float32)
        xq[:, : nq[b]] = a["q"][b][:, kept_q[b]]
        xk = np.zeros((E, Tk), np.float32)
        xk[:, : nk[b]] = a["k"][b][:, kept_k[b]]
        xv = np.zeros((E, Tk), np.float32)
        xv[:, : nk[b]] = (a["v"][b] * a["value_mask"][b][None, :].astype(np.float32))[
            :, kept_k[b]
        ]
        d["xq"], d["xk"], d["xv"] = xq, xk, xv

        for wn, key in (("wq", "Wq"), ("wk", "Wk"), ("wv", "Wv")):
            blk = wsn[key][rs : rs + 128]          # [128, E]
            blk = (i128 - m2) @ blk                # fold per-head centering
            wt = np.empty((128, E), np.float32)
            for k in range(NK):
                wt[:, 128 * k : 128 * (k + 1)] = blk[:, 128 * k : 128 * (k + 1)].T
            d[wn] = wt
        wo = np.empty((128, E), np.float32)
        for m in range(4):
            wo[:, 128 * m : 128 * (m + 1)] = wsn["Wo"][128 * m : 128 * (m + 1),
                                                       rs : rs + 128].T
        d["woT"] = wo

        flat = np.zeros(Tk, np.float32)
        flat[nk[b] :] = -BIG
        d["mbias"] = flat.reshape(NIK, 128).T  # mbias[p, i] = bias at tk=128*i+p
        d["m2b"] = m2
        d["identr"] = i128
        d["ones64"] = np.ones((1, DH), np.float32)

        if use_bias:
            for bn, key in (("bcq", "bq"), ("bck", "bk"), ("bcv", "bv")):
                bb = a[key].astype(np.float32)[rs : rs + 128]
                bc = bb - m2 @ bb
                d[bn] = bc[:, None]
        if use_affine:
            # q's cscale stays 1/SCALE; so out_q = (LN/SCALE)*g + b/SCALE
            d["geq"] = np.tile(a["ln_gq"], HPC)[:, None]
            d["beq"] = (np.tile(a["ln_bq"], HPC) / SCALE)[:, None]
            d["gek"] = np.tile(a["ln_gk"], HPC)[:, None]
            d["bek"] = np.tile(a["ln_bk"], HPC)[:, None]
            d["gev"] = np.tile(a["ln_gv"], HPC)[:, None]
            d["bev"] = np.tile(a["ln_bv"], HPC)[:, None]
        # dtype conversion: bf16 for tensors declared BF, fp32 otherwise
        for k in ("xq", "xk", "xv", "wq", "wk", "wv", "woT", "m2b"):
            d[k] = np.ascontiguousarray(d[k]).astype(BF_NP)
        for k in ("mbias", "ones64", "identr", "bcq", "bck", "bcv",
                  "geq", "beq", "gek", "bek", "gev", "bev"):
            if k in d:
                d[k] = np.ascontiguousarray(d[k], dtype=np.float32)
        in_maps.append(d)

    res = bass_utils.run_bass_kernel_spmd(
        nc,
        in_maps,
        core_ids=list(range(NCORES)),
        trace=os.environ.get("KTRACE", "0") == "1",
    )
    _last_results = res
    kernel._last_meta = {"Tq": Tq, "Tk": Tk, "nq": nq, "nk": nk,
                         "kept_q": kept_q, "kept_k": kept_k}

    out = np.zeros((B, E, T), np.float32)
    bo = a["bo"].astype(np.float32)
    for b in range(B):
        acc = res.results[G * b]["out"].astype(np.float32)
        for c in range(G * b + 1, G * b + G):
            acc = acc + res.results[c]["out"].astype(np.float32)
        out[b][:, kept_q[b]] = acc[:, : nq[b]]
        out[b] += bo[:, None]
    return out


# revision 52
# speedup vs baseline: 1.8498x; 1.0448x over previous
"""nn_MultiHeadAttention_84954453115654 — Trainium2 Bass kernel, 8 NeuronCores.

Sharding: data-parallel over batch (2) x head-pair-parallel (4 groups of 2
heads).  Core c handles batch b = c//4 and embed rows [128*(c%4), +128)
(= heads 2*(c%4), 2*(c%4)+1).  Host sums the 4 out_proj partials per batch
and adds bo (row-parallel all-reduce done at gather time).

Key idea: the masks are inputs, so the host COMPACTS the time axis before
launch.  Only unmasked query columns (Tq' ~ 1024 of 2048) and unmasked key
columns (Tk' ~ 1024) are shipped/computed; value_mask is pre-applied to the
compacted xv.  Masked query columns of the output are exactly bo (reference:
scores row zeroed -> av col 0 -> conv of masked col -> bias), so the host
scatters computed columns back and fills the rest with bo.  This halves DMA
and PE work and quarters the softmax-exp ACT work vs. the dense formulation.

Weight prep happens on host (weights are inputs): weight-standardize,
fold the per-head centering (I - M2) into Wq/Wk/Wv so the projection
matmul directly yields diff = p - mean_head(p), transpose into lhsT
layout, cast bf16.

Per core device program:
  - q/k/v projections (bf16 matmuls, K-chained over 4 tiles of 128) with
    fused per-head LayerNorm: var via block-mean matmul of diff^2, then
    out = (diff * cscale) / sqrt(var + eps) in one DVE op (1/SCALE folded
    into q's cscale)
  - scores transposed per tk-tile i: S^T[tk, tq] = kn_i^T @ qn, one
    [128, Tq] PSUM tile per (i, head); softmax exp on ACT with a
    per-partition bias (-80 on compact-pad keys, else 0); no row-max
    subtraction (post-LN scores are O(0.1))
  - av[65, 512] += vaugT_i @ exp accumulated over i in PSUM; vaug carries a
    ones column so row 64 is the softmax denominator
  - normalize: denominator row -> SBUF, PE-broadcast over 64 partitions,
    avn = av / bcast (no query-mask term: all compacted queries are live)
  - out_proj partial: woT[:, m] @ avn -> [512, Tq] stored bf16
"""

import os
import sys
import contextlib
import functools

for _p in ("/root/.axon_site/_ro/trn_rl_repo", "/opt/trn_rl_repo"):
    if os.path.isdir(_p) and _p not in sys.path:
        sys.path.append(_p)

import numpy as np
import ml_dtypes

import concourse.bass as bass
import concourse.mybir as mybir
import concourse.tile as tile
from concourse import bass_utils

B, E, T, H = 2, 512, 2048, 8
DH = E // H            # 64
HPC = 2                # heads per core
G = 4                  # cores per batch group
NCORES = 8
NK = E // 128          # 4 contraction tiles for the projections
EPS = 1e-5
SCALE = float(E // H ** 0.5)   # 181.0
BIG = 80.0
FP = mybir.dt.float32
FR = mybir.dt.float32r
BF = mybir.dt.bfloat16
AF = mybir.ActivationFunctionType
OP = mybir.AluOpType
BF_NP = np.dtype(ml_dtypes.bfloat16)


def _split_multiwaits(nc):
    """Split multi-wait instructions (Tile's tail drain) into single-wait
    EventSemaphore chains; this container's walrus encodes only one sync
    wait per instruction."""
    import bass_rust

    n_new = 0
    for f in nc.m.functions:
        for bb in f.blocks:
            out = []
            changed = False
            for ins in bb.instructions:
                si = ins.sync_info
                if si is not None and si.on_wait is not None and len(si.on_wait) > 1:
                    waits = list(si.on_wait)
                    for w in waits[:-1]:
                        ev = bass_rust.InstEventSemaphore(
                            name=f"MWFIX-{n_new}", ins=[], outs=[]
                        )
                        n_new += 1
                        ev.engine = ins.engine
                        ev.sync_info = bass_rust.SyncInfo(on_wait=[w], on_update=[])
                        out.append(ev)
                    ins.sync_info = bass_rust.SyncInfo(
                        on_wait=[waits[-1]], on_update=list(si.on_update or [])
                    )
                    changed = True
                out.append(ins)
            if changed:
                bb.instructions = out
    return n_new


def _chunks(total, step=512):
    """[(off, width), ...] covering `total` in steps of `step`."""
    out = []
    off = 0
    while off < total:
        out.append((off, min(step, total - off)))
        off += step
    return out


def _emit(nc, tc, dram, flags, dbg, Tq, Tk):
    v = nc.vector
    sc = nc.scalar
    te = nc.tensor
    gp = nc.gpsimd
    sy = nc.sync

    NIK = Tk // 128          # tk tiles
    JQ = _chunks(Tq)         # tq chunks (512-wide, last may be ragged)
    JK = _chunks(Tk)
    JT = _chunks(Tq, 1024)   # scores/exp tiles (<= 2 PSUM banks each)
    use_bias = flags["use_bias"]
    use_affine = flags["use_affine"]

    stack = contextlib.ExitStack()

    consts = stack.enter_context(tc.tile_pool(name="consts", bufs=1))
    xp = stack.enter_context(tc.tile_pool(name="x", bufs=NK))
    nbuf = stack.enter_context(tc.tile_pool(name="named", bufs=1))
    ntp = stack.enter_context(
        tc.tile_pool(name="nt", bufs=NIK * HPC * len(_chunks(Tq, 1024)) + 2)
    )
    sdp = stack.enter_context(tc.tile_pool(name="sd", bufs=3))
    dnp = stack.enter_context(tc.tile_pool(name="dn", bufs=4))
    outp = stack.enter_context(tc.tile_pool(name="outsb", bufs=4))

    # ---- constant / weight loads (DMA queue order matters) -------------
    wT = {}
    for wname in ("wq", "wk", "wv"):
        wt = consts.tile([128, E], BF, tag=wname, name=wname)
        sy.dma_start(wt[:], dram[wname])
        wT[wname] = wt
    m2b = consts.tile([128, 128], BF, tag="m2b")
    sy.dma_start(m2b[:], dram["m2b"])
    identr = consts.tile([128, 128], FR, tag="identr")
    sy.dma_start(identr[:], dram["identr"])
    ones64 = consts.tile([1, DH], FR, tag="ones64")
    sy.dma_start(ones64[:], dram["ones64"])
    mbias = consts.tile([128, NIK], FP, tag="mbias")
    sy.dma_start(mbias[:], dram["mbias"])
    epst = consts.tile([128, 1], FP, tag="eps")
    v.memset(epst[:], EPS)

    bias_tiles = {}
    if use_bias:
        for bn in ("bcq", "bck", "bcv"):
            bt = consts.tile([128, 1], FP, tag=bn, name=bn)
            sy.dma_start(bt[:], dram[bn])
            bias_tiles[bn] = bt
    ge_tiles = {}
    if use_affine:
        for gn in ("geq", "beq", "gek", "bek", "gev", "bev"):
            gt = consts.tile([128, 1], FP, tag=gn, name=gn)
            sy.dma_start(gt[:], dram[gn])
            ge_tiles[gn] = gt

    # ---- x loads: one DMA per k-tile (amortizes per-DMA fixed cost);
    # xq first (first exp needs all of qn), then xk, xv
    xts = {}
    for tn, Tx in (("xq", Tq), ("xk", Tk), ("xv", Tk)):
        tiles = {}
        for k in range(NK):
            xt = xp.tile([128, Tx], BF, tag=f"x_{tn}", name=f"x_{tn}_{k}")
            sy.dma_start(xt[:], dram[tn][128 * k : 128 * (k + 1), :])
            tiles[k] = xt
        xts[tn] = tiles
    woT = consts.tile([128, E], BF, tag="woT")
    sy.dma_start(woT[:], dram["woT"])

    # named projection outputs
    qn = nbuf.tile([128, Tq], BF, tag="qn", name="qn")
    kn = nbuf.tile([128, Tk], BF, tag="kn", name="kn")
    vn = nbuf.tile([128, Tk], FR, tag="vn", name="vn")  # fp32r: feeds PE transpose
    tn_tiles = {"xq": qn, "xk": kn, "xv": vn}
    # vaugT: per (i, h) a [128, 65] block: cols 0..63 = v^T, col 64 = ones
    vaug = nbuf.tile([128, NIK * HPC * 65], BF, tag="vaug", name="vaug")
    vaug3 = vaug[:].rearrange("p (n c) -> p n c", c=65)
    gp.memset(vaug[:], 1.0)
    avn = nbuf.tile([128, Tq], BF, tag="avn", name="avn")

    # PSUM pool stack: ps at the bottom (lives through normalize/out_proj,
    # whose rbp/po tiles ride the same tag ring), pj on top (released once
    # all projections are emitted), then pav groups.
    ps_pool = tc.alloc_tile_pool(name="ps", bufs=2, space="PSUM")
    pj = tc.alloc_tile_pool(name="pj", bufs=2, space="PSUM")

    def project_chunk(tn, wname, j, joff, jw, cscale, bn, gn, bln, dsq_eng):
        """Project x chunk j and apply per-head LN; writes tn_tiles[tn]."""
        out = tn_tiles[tn]
        pp = pj.tile([128, 512], FP, tag="pp", name=f"pp_{tn}_{j}")
        for k in range(NK):
            te.matmul(
                pp[:, :jw],
                wT[wname][:, 128 * k : 128 * (k + 1)],
                xts[tn][k][:, joff : joff + jw],
                start=(k == 0),
                stop=(k == NK - 1),
            )
        if use_bias or dsq_eng is not sc:
            # SBUF copy of diff: PSUM-reading two-input ops are illegal, so
            # non-ACT squares (and the bias add) go through SBUF
            u = sdp.tile([128, 512], FP, tag="u", name=f"u_{tn}_{j}")
            if use_bias:
                v.tensor_scalar_add(u[:, :jw], pp[:, :jw], bias_tiles[bn][:])
            else:
                v.tensor_scalar_add(u[:, :jw], pp[:, :jw], 0.0)
            diff = u
        else:
            diff = pp
        dsq = sdp.tile([128, 512], BF, tag="dsq", name=f"dsq_{tn}_{j}")
        if dsq_eng is sc:
            sc.activation(dsq[:, :jw], diff[:, :jw], AF.Square)
        else:
            dsq_eng.tensor_mul(dsq[:, :jw], diff[:, :jw], diff[:, :jw])
        pvar = pj.tile([128, 512], FP, tag="pvar", bufs=1, name=f"pvar_{tn}_{j}")
        te.matmul(pvar[:, :jw], m2b[:], dsq[:, :jw], start=True, stop=True)
        sd = sdp.tile([128, 512], FP, tag="sd", name=f"sd_{tn}_{j}")
        sc.activation(sd[:, :jw], pvar[:, :jw], AF.Sqrt, bias=epst[:])
        rsq = sdp.tile([128, 512], FP, tag="rsq", name=f"rsq_{tn}_{j}")
        v.reciprocal(rsq[:, :jw], sd[:, :jw])
        v.scalar_tensor_tensor(
            out[:, joff : joff + jw],
            diff[:, :jw],
            cscale,
            rsq[:, :jw],
            op0=OP.mult,
            op1=OP.mult,
        )
        if use_affine:
            v.tensor_scalar(
                out[:, joff : joff + jw],
                out[:, joff : joff + jw],
                ge_tiles[gn][:],
                ge_tiles[bln][:],
                op0=OP.mult,
                op1=OP.add,
            )

    def vchunk(j, joff, jw):
        """v projection chunk + transposes of its tk tiles into vaug."""
        project_chunk("xv", "wv", j, joff, jw, 1.0, "bcv", "gev", "bev", gp)
        for ii in range(jw // 128):
            i = joff // 128 + ii
            pt = pj.tile([128, 128], FR, tag="ptr", bufs=1, name=f"ptr{i}")
            te.transpose(pt[:], vn[:, 128 * i : 128 * (i + 1)], identr[:])
            # both heads in one strided copy: [128, (h), 64] -> vaug blocks
            src = pt[:].bitcast(FP).rearrange("p (h c) -> p h c", c=DH)
            dst = vaug3[:, HPC * i : HPC * i + HPC, 0:DH]
            v.tensor_copy(dst, src)

    # ---- emission: q proj, k proj, then attention with v interleaved ---
    for j, (joff, jw) in enumerate(JQ):
        project_chunk("xq", "wq", j, joff, jw, 1.0 / SCALE, "bcq", "geq", "beq", sc)
    for j, (joff, jw) in enumerate(JK):
        project_chunk("xk", "wk", j, joff, jw, 1.0, "bck", "gek", "bek", sc)

    # ---- attention ----------------------------------------------------
    # per (i, h): one [128, Tq] scores tile -> exp -> nt (SBUF); av chains
    # for the first JQ group stream behind the exps once pj's banks free up
    groups = [list(enumerate(JQ))[g0 : g0 + 2] for g0 in range(0, len(JQ), 2)]

    def make_av_tiles(pav, grp):
        return {
            (h, j): pav.tile([DH + 1, jw], FP, tag=f"av{h}_{j}", name=f"av{h}_{j}")
            for h in range(HPC)
            for j, (joff, jw) in grp
        }

    def av_step(av_tiles, grp, i):
        for h in range(HPC):
            for j, (joff, jw) in grp:
                jt, off = joff // 1024, joff % 1024
                te.matmul(
                    av_tiles[(h, j)][:],
                    vaug3[:, HPC * i + h, :],
                    nts[(i, h, jt)][:, off : off + jw],
                    start=(i == 0),
                    stop=(i == NIK - 1),
                )

    nts = {}
    vi = 0  # next v chunk to emit
    pav = None
    for i in range(NIK):
        # interleave v projection chunks so the PE wait on xv DMA lands
        # between scores the ACT stream has already consumed
        if i >= 2 and i % 2 == 0 and vi < len(JK):
            vchunk(vi, *JK[vi])
            vi += 1
        for h in range(HPC):
            hs = slice(DH * h, DH * (h + 1))
            for jt, (toff, tw) in enumerate(JT):
                ps = ps_pool.tile([128, 1024], FP, tag="ps", name=f"ps{i}_{h}_{jt}")
                for soff, sw in _chunks(tw):
                    te.matmul(
                        ps[:, soff : soff + sw],
                        kn[hs, 128 * i : 128 * (i + 1)],
                        qn[hs, toff + soff : toff + soff + sw],
                        start=True,
                        stop=True,
                    )
                nt = ntp.tile([128, 1024], BF, tag="nt", name=f"nt{i}_{h}_{jt}")
                sc.activation(nt[:, :tw], ps[:, :tw], AF.Exp, bias=mbias[:, i : i + 1])
                nts[(i, h, jt)] = nt
        if pav is None and vi == len(JK):
            pj.release()
            pav = tc.alloc_tile_pool(name="pav0", bufs=1, space="PSUM")
            av_tiles = make_av_tiles(pav, groups[0])
            for ii in range(i + 1):
                av_step(av_tiles, groups[0], ii)
        elif pav is not None:
            av_step(av_tiles, groups[0], i)
    if pav is None:
        while vi < len(JK):
            vchunk(vi, *JK[vi])
            vi += 1
        pj.release()
        pav = tc.alloc_tile_pool(name="pav0", bufs=1, space="PSUM")
        av_tiles = make_av_tiles(pav, groups[0])
        for ii in range(NIK):
            av_step(av_tiles, groups[0], ii)

    # ---- normalize + out_proj + store ---------------------------------
    # rbp/po PSUM tiles ride the ps pool's tag ring (banks free post-exp)
    def norm_and_out(grp, av_tiles):
        for idx, (j, (joff, jw)) in enumerate(
            (j, jj) for j, jj in grp for _ in range(HPC)
        ):
            h = idx % HPC
            if True:
                hs = slice(DH * h, DH * (h + 1))
                avt = av_tiles[(h, j)]
                avsb = dnp.tile([DH, 512], FP, tag="avsb", name=f"avsb{h}_{j}")
                dnm = dnp.tile([1, 512], FP, tag="dnm", name=f"dnm{h}_{j}")
                if idx % 2 == 0:
                    v.tensor_copy(avsb[:, :jw], avt[0:DH, :])
                    sc.copy(dnm[:, :jw], avt[DH : DH + 1, :])
                else:
                    sc.copy(avsb[:, :jw], avt[0:DH, :])
                    v.tensor_copy(dnm[:, :jw], avt[DH : DH + 1, :])
                rdn = dnp.tile([1, 512], FR, tag="rdn", name=f"rdn{h}_{j}")
                with nc.allow_low_precision(reason="fp32r broadcast of recip row"):
                    v.reciprocal(rdn[:, :jw], dnm[:, :jw])
                rbp = ps_pool.tile([128, 1024], FP, tag="ps", name=f"rbp{h}_{j}")
                te.matmul(
                    rbp[0:DH, :jw],
                    ones64[:],
                    rdn[:, :jw],
                    start=True,
                    stop=True,
                )
                v.tensor_mul(
                    avn[hs, joff : joff + jw],
                    avsb[:, :jw],
                    rbp[0:DH, :jw],
                )
        for j, (joff, jw) in grp:
            for m in range(4):
                po = ps_pool.tile([128, 1024], FP, tag="ps", name=f"po{j}_{m}")
                te.matmul(
                    po[:, :jw],
                    woT[:, 128 * m : 128 * (m + 1)],
                    avn[:, joff : joff + jw],
                    start=True,
                    stop=True,
                )
                ot = outp.tile([128, 512], BF, tag="ot", name=f"ot{j}_{m}")
                if m % 2 == 0:
                    v.tensor_copy(ot[:, :jw], po[:, :jw])
                else:
                    sc.copy(ot[:, :jw], po[:, :jw])
                sy.dma_start(
                    dram["out"][128 * m : 128 * (m + 1), joff : joff + jw],
                    ot[:, :jw],
                )

    norm_and_out(groups[0], av_tiles)
    pav.release()
    for gi, grp in enumerate(groups[1:], 1):
        pav = tc.alloc_tile_pool(name=f"pav{gi}", bufs=1, space="PSUM")
        av_tiles = make_av_tiles(pav, grp)
        for ii in range(NIK):
            av_step(av_tiles, grp, ii)
        norm_and_out(grp, av_tiles)
        pav.release()
    ps_pool.release()

    for dname in dbg:
        src = {"qn": qn, "kn": kn, "vn": vn, "avn": avn}[dname]
        sy.dma_start(dram["dbg_" + dname][:, : src.shape[1]], src[:])

    stack.close()


_last_dims = (1024, 1024)


def _build(use_bias, use_affine, debug_names="", Tq=None, Tk=None):
    if Tq is None or Tk is None:
        Tq, Tk = _last_dims
    return _build_impl(use_bias, use_affine, debug_names, Tq, Tk)


@functools.lru_cache(maxsize=4)
def _build_impl(use_bias, use_affine, debug_names, Tq, Tk):
    nc = bass.Bass(
        "TRN2", target_bir_lowering=False, debug=False, num_devices=NCORES
    )
    NIK = Tk // 128
    dram = {}
    dram["xq"] = nc.dram_tensor("xq", [E, Tq], BF, kind="ExternalInput").ap()
    dram["xk"] = nc.dram_tensor("xk", [E, Tk], BF, kind="ExternalInput").ap()
    dram["xv"] = nc.dram_tensor("xv", [E, Tk], BF, kind="ExternalInput").ap()
    for wn in ("wq", "wk", "wv", "woT"):
        dram[wn] = nc.dram_tensor(wn, [128, E], BF, kind="ExternalInput").ap()
    dram["m2b"] = nc.dram_tensor("m2b", [128, 128], BF, kind="ExternalInput").ap()
    dram["identr"] = nc.dram_tensor("identr", [128, 128], FR, kind="ExternalInput").ap()
    dram["ones64"] = nc.dram_tensor("ones64", [1, DH], FR, kind="ExternalInput").ap()
    dram["mbias"] = nc.dram_tensor("mbias", [128, NIK], FP, kind="ExternalInput").ap()
    if use_bias:
        for bn in ("bcq", "bck", "bcv"):
            dram[bn] = nc.dram_tensor(bn, [128, 1], FP, kind="ExternalInput").ap()
    if use_affine:
        for gn in ("geq", "beq", "gek", "bek", "gev", "bev"):
            dram[gn] = nc.dram_tensor(gn, [128, 1], FP, kind="ExternalInput").ap()
    dram["out"] = nc.dram_tensor("out", [E, Tq], BF, kind="ExternalOutput").ap()
    dbg = frozenset(debug_names.split(",")) - {""} if debug_names else frozenset()
    for dname in dbg:
        w = Tq if dname in ("qn", "avn") else Tk
        dram["dbg_" + dname] = nc.dram_tensor(
            "dbg_" + dname, [128, w], BF, kind="ExternalOutput"
        ).ap()

    flags = {"use_bias": use_bias, "use_affine": use_affine}
    with tile.TileContext(nc) as tc:
        _emit(nc, tc, dram, flags, dbg, Tq, Tk)
    _split_multiwaits(nc)
    return nc


def _pad_up(n, m):
    return max(m, ((n + m - 1) // m) * m)


@functools.lru_cache(maxsize=1)
def _m2_const():
    m2 = np.zeros((128, 128), np.float32)
    m2[:DH, :DH] = 1.0 / DH
    m2[DH:, DH:] = 1.0 / DH
    return m2


def _std(w):
    mu = w.mean(axis=1, keepdims=True)
    var = w.var(axis=1, keepdims=True)
    return (w - mu) / np.sqrt(var + EPS)


_last_results = None


def kernel(**inputs):
    global _last_results
    a = {k: np.asarray(val) for k, val in inputs.items()}
    use_bias = bool(any(np.any(a[bn] != 0) for bn in ("bq", "bk", "bv")))
    use_affine = bool(
        any(np.any(a[gn] != 1) for gn in ("ln_gq", "ln_gk", "ln_gv"))
        or any(np.any(a[bn] != 0) for bn in ("ln_bq", "ln_bk", "ln_bv"))
    )
    debug_names = os.environ.get("KDEBUG", "")

    qm = a["query_mask"].astype(bool)
    km = a["key_mask"].astype(bool)
    kept_q = [np.flatnonzero(qm[b]) for b in range(B)]
    kept_k = [np.flatnonzero(km[b]) for b in range(B)]
    nq = [len(ix) for ix in kept_q]
    nk = [len(ix) for ix in kept_k]
    Tq = _pad_up(max(nq), 128)
    Tk = _pad_up(max(nk), 128)
    NIK = Tk // 128

    global _last_dims
    _last_dims = (Tq, Tk)
    nc = _build(use_bias, use_affine, debug_names, Tq, Tk)

    # host weight prep (fp32), shared across cores of the same head group
    m2 = _m2_const()
    i128 = np.eye(128, dtype=np.float32)
    wsn = {wn: _std(a[wn].astype(np.float32)) for wn in ("Wq", "Wk", "Wv", "Wo")}

    in_maps = []
    for c in range(NCORES):
        b, hp = divmod(c, G)
        rs = 128 * hp
        d = {}
        # compact + pad x; value_mask pre-applied to xv
        xq = np.zeros((E, Tq), np.float32)
        xq[:, : nq[b]] = a["q"][b][:, kept_q[b]]
        xk = np.zeros((E, Tk), np.float32)
        xk[:, : nk[b]] = a["k"][b][:, kept_k[b]]
        xv = np.zeros((E, Tk), np.float32)
        xv[:, : nk[b]] = (a["v"][b] * a["value_mask"][b][None, :].astype(np.float32))[
            :, kept_k[b]
        ]
        d["xq"], d["xk"], d["xv"] = xq, xk, xv

        for wn, key in (("wq", "Wq"), ("wk", "Wk"), ("wv", "Wv")):
            blk = wsn[key][rs : rs + 128]          # [128, E]
            blk = (i128 - m2) @ blk                # fold per-head centering
            wt = np.empty((128, E), np.float32)
            for k in range(NK):
                wt[:, 128 * k : 128 * (k + 1)] = blk[:, 128 * k : 128 * (k + 1)].T
            d[wn] = wt
        wo = np.empty((128, E), np.float32)
        for m in range(4):
            wo[:, 128 * m : 128 * (m + 1)] = wsn["Wo"][128 * m : 128 * (m + 1),
                                                       rs : rs + 128].T
        d["woT"] = wo

        flat = np.zeros(Tk, np.float32)
        flat[nk[b] :] = -BIG
        d["mbias"] = flat.reshape(NIK, 128).T  # mbias[p, i] = bias at tk=128*i+p
        d["m2b"] = m2
        d["identr"] = i128
        d["ones64"] = np.ones((1, DH), np.float32)

        if use_bias:
            for bn, key in (("bcq", "bq"), ("bck", "bk"), ("bcv", "bv")):
                bb = a[key].astype(np.float32)[rs : rs + 128]
                bc = bb - m2 @ bb
                d[bn] = bc[:, None]
        if use_affine:
            # q's cscale stays 1/SCALE; so out_q = (LN/SCALE)*g + b/SCALE
            d["geq"] = np.tile(a["ln_gq"], HPC)[:, None]
            d["beq"] = (np.tile(a["ln_bq"], HPC) / SCALE)[:, None]
            d["gek"] = np.tile(a["ln_gk"], HPC)[:, None]
            d["bek"] = np.tile(a["ln_bk"], HPC)[:, None]
            d["gev"] = np.tile(a["ln_gv"], HPC)[:, None]
            d["bev"] = np.tile(a["ln_bv"], HPC)[:, None]
        # dtype conversion: bf16 for tensors declared BF, fp32 otherwise
        for k in ("xq", "xk", "xv", "wq", "wk", "wv", "woT", "m2b"):
            d[k] = np.ascontiguousarray(d[k]).astype(BF_NP)
        for k in ("mbias", "ones64", "identr", "bcq", "bck", "bcv",
                  "geq", "beq", "gek", "bek", "gev", "bev"):
            if k in d:
                d[k] = np.ascontiguousarray(d[k], dtype=np.float32)
        in_maps.append(d)

    res = bass_utils.run_bass_kernel_spmd(
        nc,
        in_maps,
        core_ids=list(range(NCORES)),
        trace=os.environ.get("KTRACE", "0") == "1",
    )
    _last_results = res
    kernel._last_meta = {"Tq": Tq, "Tk": Tk, "nq": nq, "nk": nk,
                         "kept_q": kept_q, "kept_k": kept_k}

    out = np.zeros((B, E, T), np.float32)
    bo = a["bo"].astype(np.float32)
    for b in range(B):
        acc = res.results[G * b]["out"].astype(np.float32)
        for c in range(G * b + 1, G * b + G):
            acc = acc + res.results[c]["out"].astype(np.float32)
        out[b][:, kept_q[b]] = acc[:, : nq[b]]
        out[b] += bo[:, None]
    return out


# revision 67
# speedup vs baseline: 2.1595x; 1.1674x over previous
"""nn_MultiHeadAttention_84954453115654 — Trainium2 Bass kernel, 8 NeuronCores.

Sharding: data-parallel over batch (2) x head-pair-parallel (4 groups of 2
heads).  Core c handles batch b = c//4 and embed rows [128*(c%4), +128)
(= heads 2*(c%4), 2*(c%4)+1).  Host sums the 4 out_proj partials per batch
and adds bo (the row-parallel all-reduce, done at gather time).

Key idea: the masks are inputs, so the host COMPACTS the time axis before
launch.  Only unmasked query columns (Tq ~ half of 2048) and unmasked key
columns (Tk ~ half) are shipped/computed; value_mask is pre-applied to the
compacted xv.  Masked query columns of the output are exactly bo (reference:
softmax scores row zeroed -> av col 0 -> out col = bias), so the host
scatters computed columns back and fills the rest with bo.  This halves DMA
and PE work and quarters the softmax-exp ACT work vs. dense.

Weight prep happens on host (weights are inputs): weight-standardize, fold
the per-head centering (I - M2) into Wq/Wk/Wv so the projection matmul
directly yields diff = p - mean_head(p), transpose into lhsT layout, bf16.

Per-core device program:
  - q/k/v projections (bf16 matmuls, K-chained over 4 tiles of 128) with
    fused per-head LayerNorm: var via block-mean matmul of diff^2, then
    out = (diff * cscale) * reciprocal(sqrt(var + eps)) (1/SCALE folded
    into q's cscale)
  - scores transposed per (tk-tile i, head, tq-tile): S^T[tk, tq] =
    kn_i^T @ qn chunk; softmax exp on ACT with a per-partition bias (-80
    on compact-pad keys, else 0); no row-max subtraction (post-LN scores
    are O(0.1) so exp cannot overflow)
  - av[65, jw] += vaugT_i @ exp accumulated over i in PSUM; vaug carries a
    ones column so row 64 is the softmax denominator; av matmuls are
    spread between exp units to keep the ACT stream dense
  - normalize: denominator rows batched into one tile, one reciprocal,
    PE-broadcast per row, avn = avsb * bcast
  - out_proj partial: woT[:, m] @ avn -> PSUM, DMA'd straight to DRAM
    (bf16 cast in the DMA, via gpsimd)
"""

import os
import sys
import contextlib
import functools

for _p in ("/root/.axon_site/_ro/trn_rl_repo", "/opt/trn_rl_repo"):
    if os.path.isdir(_p) and _p not in sys.path:
        sys.path.append(_p)

import numpy as np
import ml_dtypes

import concourse.bass as bass
import concourse.mybir as mybir
import concourse.tile as tile
from concourse import bass_utils

B, E, T, H = 2, 512, 2048, 8
DH = E // H            # 64
HPC = 2                # heads per core
G = 4                  # cores per batch group
NCORES = 8
NK = E // 128          # 4 contraction tiles for the projections
EPS = 1e-5
SCALE = float(E // H ** 0.5)   # 181.0
BIG = 80.0
FP = mybir.dt.float32
FR = mybir.dt.float32r
BF = mybir.dt.bfloat16
AF = mybir.ActivationFunctionType
OP = mybir.AluOpType
BF_NP = np.dtype(ml_dtypes.bfloat16)


def _split_multiwaits(nc):
    """Split multi-wait instructions (Tile's tail drain) into single-wait
    EventSemaphore chains; this container's walrus encodes only one sync
    wait per instruction."""
    import bass_rust

    n_new = 0
    for f in nc.m.functions:
        for bb in f.blocks:
            out = []
            changed = False
            for ins in bb.instructions:
                si = ins.sync_info
                if si is not None and si.on_wait is not None and len(si.on_wait) > 1:
                    waits = list(si.on_wait)
                    for w in waits[:-1]:
                        ev = bass_rust.InstEventSemaphore(
                            name=f"MWFIX-{n_new}", ins=[], outs=[]
                        )
                        n_new += 1
                        ev.engine = ins.engine
                        ev.sync_info = bass_rust.SyncInfo(on_wait=[w], on_update=[])
                        out.append(ev)
                    ins.sync_info = bass_rust.SyncInfo(
                        on_wait=[waits[-1]], on_update=list(si.on_update or [])
                    )
                    changed = True
                out.append(ins)
            if changed:
                bb.instructions = out
    return n_new


def _chunks(total, step=512):
    out = []
    off = 0
    while off < total:
        out.append((off, min(step, total - off)))
        off += step
    return out


def _jt_split(Tq):
    """scores/exp tile widths: one tile if it fits 2 PSUM banks, else
    (512, rest) so the first exp only needs the first qn chunk."""
    if Tq <= 1024:
        return [(0, Tq)]
    assert Tq <= 1536, f"Tq={Tq} too large for the 2-tile scores layout"
    return [(0, 512), (512, Tq - 512)]


def _emit(nc, tc, dram, flags, dbg, Tq, Tk):
    v = nc.vector
    sc = nc.scalar
    te = nc.tensor
    gp = nc.gpsimd
    sy = nc.sync

    NIK = Tk // 128          # tk tiles
    JQ = _chunks(Tq)         # tq chunks (512-wide, last may be ragged)
    JK = _chunks(Tk)
    JT = _jt_split(Tq)
    use_bias = flags["use_bias"]
    use_affine = flags["use_affine"]

    stack = contextlib.ExitStack()

    consts = stack.enter_context(tc.tile_pool(name="consts", bufs=1))
    xp = stack.enter_context(tc.tile_pool(name="x", bufs=NK))
    xcp = stack.enter_context(tc.tile_pool(name="xc", bufs=2))
    nbuf = stack.enter_context(tc.tile_pool(name="named", bufs=1))
    ntp = stack.enter_context(
        tc.tile_pool(name="nt", bufs=(Tk // 128) * HPC * len(_jt_split(Tq)) + 2)
    )
    sdp = stack.enter_context(tc.tile_pool(name="sd", bufs=3))
    dnp = stack.enter_context(tc.tile_pool(name="dn", bufs=1))

    # ---- packed constant loads (2 DMAs) --------------------------------
    # wpack: [wq | wk | wv | woT | m2b] bf16; cpack: [identr | mbias | onesP]
    wpack = consts.tile([128, 4 * E + 128], BF, tag="wpack")
    sy.dma_start(wpack[:], dram["wpack"])
    wT = {n: wpack[:, i * E : (i + 1) * E] for i, n in enumerate(("wq", "wk", "wv"))}
    woT = wpack[:, 3 * E : 4 * E]
    m2b = wpack[:, 4 * E : 4 * E + 128]

    cpackr = consts.tile([128, 128 + DH], FR, tag="cpackr")
    sy.dma_start(cpackr[:], dram["cpackr"])
    identr = cpackr[:, 0:128]
    onesP = cpackr[:, 128 : 128 + DH]  # all-ones rows (base-aligned lhsT)
    mbias = consts.tile([128, NIK], FP, tag="mbias")
    sy.dma_start(mbias[:], dram["mbias"])
    epst = consts.tile([128, 1], FP, tag="eps")
    v.memset(epst[:], EPS)

    bias_tiles = {}
    if use_bias:
        for bn in ("bcq", "bck", "bcv"):
            bt = consts.tile([128, 1], FP, tag=bn, name=bn)
            sy.dma_start(bt[:], dram[bn])
            bias_tiles[bn] = bt
    ge_tiles = {}
    if use_affine:
        for gn in ("geq", "beq", "gek", "bek", "gev", "bev"):
            gt = consts.tile([128, 1], FP, tag=gn, name=gn)
            sy.dma_start(gt[:], dram[gn])
            ge_tiles[gn] = gt

    # ---- x loads -------------------------------------------------------
    # xq: one DMA per k-tile (full rows needed before any exp); xk/xv: one
    # DMA per (512-col chunk, k-tile), chunk-major, so kn chunk 0 unlocks
    # the first scores as early as possible.
    xq_tiles = {}
    for k in range(NK):
        xt = xp.tile([128, Tq], BF, tag="x_xq", name=f"x_xq_{k}")
        sy.dma_start(xt[:], dram["xq"][128 * k : 128 * (k + 1), :])
        xq_tiles[k] = xt
    xc_tiles = {}
    for j, (joff, jw) in enumerate(JK):
        for tn in ("xk", "xv"):
            for k in range(NK):
                xt = xcp.tile([128, 512], BF, tag=f"xc_{tn}_{k}", name=f"x_{tn}_{k}_{j}")
                sy.dma_start(
                    xt[:, :jw], dram[tn][128 * k : 128 * (k + 1), joff : joff + jw]
                )
                xc_tiles[(tn, k, j)] = xt

    # named projection outputs
    qn = nbuf.tile([128, Tq], BF, tag="qn", name="qn")
    kn = nbuf.tile([128, Tk], BF, tag="kn", name="kn")
    vn = nbuf.tile([128, Tk], FR, tag="vn", name="vn")  # fp32r: feeds transpose
    # vaugT: per (i, h) a [128, 65] block: cols 0..63 = v^T, col 64 = ones
    vaug = nbuf.tile([128, NIK * HPC * 65], BF, tag="vaug", name="vaug")
    vaug3 = vaug[:].rearrange("p (n c) -> p n c", c=65)
    gp.memset(vaug[:], 1.0)
    avn = nbuf.tile([128, Tq], BF, tag="avn", name="avn")

    # PSUM pool stack: ps at the bottom (also hosts rbp/po at the tail),
    # pj on top (released once all projections are emitted), then pav.
    ps_pool = tc.alloc_tile_pool(name="ps", bufs=2, space="PSUM")
    pj = tc.alloc_tile_pool(name="pj", bufs=2, space="PSUM")

    def project_chunk(tn, wname, joff, jw, cscale, bn, gn, bln, dsq_eng):
        """Project x cols [joff, joff+jw) and apply per-head LN."""
        out = {"xq": qn, "xk": kn, "xv": vn}[tn]
        j = joff // 512
        pp = pj.tile([128, 512], FP, tag="pp", name=f"pp_{tn}_{j}")
        for k in range(NK):
            src = (
                xq_tiles[k][:, joff : joff + jw]
                if tn == "xq"
                else xc_tiles[(tn, k, j)][:, :jw]
            )
            te.matmul(
                pp[:, :jw],
                wT[wname][:, 128 * k : 128 * (k + 1)],
                src,
                start=(k == 0),
                stop=(k == NK - 1),
            )
        if use_bias or dsq_eng is not sc:
            # SBUF copy of diff: two-PSUM-input ops are illegal, so non-ACT
            # squares (and the bias add) go through SBUF
            u = sdp.tile([128, 512], FP, tag="u", name=f"u_{tn}_{j}")
            v.tensor_scalar_add(
                u[:, :jw], pp[:, :jw], bias_tiles[bn][:] if use_bias else 0.0
            )
            diff = u
        else:
            diff = pp
        dsq = sdp.tile([128, 512], BF, tag="dsq", name=f"dsq_{tn}_{j}")
        if dsq_eng is sc:
            sc.activation(dsq[:, :jw], diff[:, :jw], AF.Square)
        else:
            dsq_eng.tensor_mul(dsq[:, :jw], diff[:, :jw], diff[:, :jw])
        pvar = pj.tile([128, 512], FP, tag="pvar", bufs=1, name=f"pvar_{tn}_{j}")
        te.matmul(pvar[:, :jw], m2b, dsq[:, :jw], start=True, stop=True)
        sd = sdp.tile([128, 512], FP, tag="sd", name=f"sd_{tn}_{j}")
        sc.activation(sd[:, :jw], pvar[:, :jw], AF.Sqrt, bias=epst[:])
        rsq = sdp.tile([128, 512], FP, tag="rsq", name=f"rsq_{tn}_{j}")
        v.reciprocal(rsq[:, :jw], sd[:, :jw])
        v.scalar_tensor_tensor(
            out[:, joff : joff + jw],
            diff[:, :jw],
            cscale,
            rsq[:, :jw],
            op0=OP.mult,
            op1=OP.mult,
        )
        if use_affine:
            v.tensor_scalar(
                out[:, joff : joff + jw],
                out[:, joff : joff + jw],
                ge_tiles[gn][:],
                ge_tiles[bln][:],
                op0=OP.mult,
                op1=OP.add,
            )

    def vchunk(joff, jw):
        """v projection chunk + transposes of its tk tiles into vaug."""
        project_chunk("xv", "wv", joff, jw, 1.0, "bcv", "gev", "bev", gp)
        for ii in range(jw // 128):
            i = joff // 128 + ii
            pt = pj.tile([128, 128], FR, tag="ptr", bufs=1, name=f"ptr{i}")
            te.transpose(pt[:], vn[:, 128 * i : 128 * (i + 1)], identr)
            src = pt[:].bitcast(FP).rearrange("p (h c) -> p h c", c=DH)
            dst = vaug3[:, HPC * i : HPC * i + HPC, 0:DH]
            v.tensor_copy(dst, src)

    # ---- emission: q proj, k chunk 0, then attention with k/v interleaved
    for joff, jw in JQ:
        project_chunk("xq", "wq", joff, jw, 1.0 / SCALE, "bcq", "geq", "beq", sc)
    project_chunk("xk", "wk", *JK[0], 1.0, "bck", "gek", "bek", sc)

    # ---- attention -----------------------------------------------------
    groups = [list(enumerate(JQ))[g0 : g0 + 2] for g0 in range(0, len(JQ), 2)]
    grp0 = groups[0]

    nts = {}
    av_pending = []
    av_tiles = {}

    def av_push(i):
        for h in range(HPC):
            for j, (joff, jw) in grp0:
                av_pending.append((i, h, j, joff, jw))

    def av_emit(limit):
        n = 0
        while av_pending and n < limit:
            i, h, j, joff, jw = av_pending.pop(0)
            jt = 0 if joff < JT[0][1] else 1
            toff = JT[jt][0]
            te.matmul(
                av_tiles[(h, j)][:],
                vaug3[:, HPC * i + h, :],
                nts[(i, h, jt)][:, joff - toff : joff - toff + jw],
                start=(i == 0),
                stop=(i == NIK - 1),
            )
            n += 1

    vi = 0
    ki = 1
    pav = None
    for i in range(NIK):
        # interleave later k/v projection chunks; spacing keeps the PE wait
        # on their DMAs behind scores the ACT stream has already consumed
        if ki < len(JK) and i == ki + 2:
            project_chunk("xk", "wk", *JK[ki], 1.0, "bck", "gek", "bek", v)
            ki += 1
        if i >= 2 and vi < len(JK):
            vchunk(*JK[vi])
            vi += 1
        for h in range(HPC):
            hs = slice(DH * h, DH * (h + 1))
            for jt, (toff, tw) in enumerate(JT):
                ps = ps_pool.tile([128, 1024], FP, tag="ps", name=f"ps{i}_{h}_{jt}")
                for soff, sw in _chunks(tw):
                    te.matmul(
                        ps[:, soff : soff + sw],
                        kn[hs, 128 * i : 128 * (i + 1)],
                        qn[hs, toff + soff : toff + soff + sw],
                        start=True,
                        stop=True,
                    )
                nt = ntp.tile([128, 1024], BF, tag="nt", name=f"nt{i}_{h}_{jt}")
                sc.activation(
                    nt[:, :tw], ps[:, :tw], AF.Exp, bias=mbias[:, i : i + 1]
                )
                nts[(i, h, jt)] = nt
            if pav is not None:
                av_emit(3)
        av_push(i)
        if pav is None and vi == len(JK) and ki == len(JK):
            pj.release()
            pav = tc.alloc_tile_pool(name="pav0", bufs=1, space="PSUM")
            for h in range(HPC):
                for j, (joff, jw) in grp0:
                    av_tiles[(h, j)] = pav.tile(
                        [DH + 1, jw], FP, tag=f"av{h}_{j}", name=f"av{h}_{j}"
                    )
    if pav is None:
        while ki < len(JK):
            project_chunk("xk", "wk", *JK[ki], 1.0, "bck", "gek", "bek", v)
            ki += 1
        while vi < len(JK):
            vchunk(*JK[vi])
            vi += 1
        pj.release()
        pav = tc.alloc_tile_pool(name="pav0", bufs=1, space="PSUM")
        for h in range(HPC):
            for j, (joff, jw) in grp0:
                av_tiles[(h, j)] = pav.tile(
                    [DH + 1, jw], FP, tag=f"av{h}_{j}", name=f"av{h}_{j}"
                )
    av_emit(10**9)

    # ---- normalize + out_proj + store ---------------------------------
    # Denominator rows are gathered into one tile per group (row r), one
    # reciprocal, then a per-row PE broadcast (onesP rows keep base
    # partitions aligned).  out_proj PSUM rides the ps tag ring and is
    # DMA'd straight to DRAM with a bf16 cast (gpsimd-initiated).
    outp = stack.enter_context(tc.tile_pool(name="outsb", bufs=4))

    def norm_and_out(grp, av_tiles):
        pairs = [(h, j, joff, jw) for j, (joff, jw) in grp for h in range(HPC)]
        # recip rows live at (partition 32*(r%2), col slot 512*(r//2)) so the
        # broadcast matmul operands sit at legal base partitions (0/32)
        rw = 512 * ((len(pairs) + 1) // 2)
        rcp = dnp.tile([128, 1024], FR, tag="rcp", name="rcp")
        rslc = lambda r, jw: rcp[
            32 * (r % 2) : 32 * (r % 2) + 1, 512 * (r // 2) : 512 * (r // 2) + jw
        ]
        avsbs = {}
        for r, (h, j, joff, jw) in enumerate(pairs):
            avt = av_tiles[(h, j)]
            avsb = dnp.tile([DH, 512], FP, tag=f"avsb{r % 4}", name=f"avsb{h}_{j}")
            avsbs[(h, j)] = avsb
            if r % 2 == 0:
                v.tensor_copy(avsb[:, :jw], avt[0:DH, :])
                sc.copy(rslc(r, jw), avt[DH : DH + 1, :])
            else:
                sc.copy(avsb[:, :jw], avt[0:DH, :])
                v.tensor_copy(rslc(r, jw), avt[DH : DH + 1, :])
        with nc.allow_low_precision(reason="fp32r recip rows for PE broadcast"):
            v.reciprocal(rcp[0:33, :rw], rcp[0:33, :rw])
        for r, (h, j, joff, jw) in enumerate(pairs):
            hs = slice(DH * h, DH * (h + 1))
            rbp = ps_pool.tile([128, 1024], FP, tag="ps", name=f"rbp{h}_{j}")
            te.matmul(
                rbp[0:DH, :jw],
                onesP[32 * (r % 2) : 32 * (r % 2) + 1, :],
                rslc(r, jw),
                start=True,
                stop=True,
            )
            v.tensor_mul(
                avn[hs, joff : joff + jw], avsbs[(h, j)][:, :jw], rbp[0:DH, :jw]
            )
        for j, (joff, jw) in grp:
            for m in range(4):
                po = ps_pool.tile([128, 1024], FP, tag="ps", name=f"po{j}_{m}")
                te.matmul(
                    po[:, :jw],
                    woT[:, 128 * m : 128 * (m + 1)],
                    avn[:, joff : joff + jw],
                    start=True,
                    stop=True,
                )
                ot = outp.tile([128, 512], BF, tag="ot", name=f"ot{j}_{m}")
                if m % 2 == 0:
                    v.tensor_copy(ot[:, :jw], po[:, :jw])
                else:
                    sc.copy(ot[:, :jw], po[:, :jw])
                sy.dma_start(
                    dram["out"][128 * m : 128 * (m + 1), joff : joff + jw],
                    ot[:, :jw],
                )

    norm_and_out(grp0, av_tiles)
    pav.release()
    for gi, grp in enumerate(groups[1:], 1):
        pav = tc.alloc_tile_pool(name=f"pav{gi}", bufs=1, space="PSUM")
        av_tiles = {
            (h, j): pav.tile([DH + 1, jw], FP, tag=f"av{h}_{j}", name=f"av{h}_{j}")
            for j, (joff, jw) in grp
            for h in range(HPC)
        }
        for i in range(NIK):
            for h in range(HPC):
                for j, (joff, jw) in grp:
                    jt = 0 if joff < JT[0][1] else 1
                    toff = JT[jt][0]
                    te.matmul(
                        av_tiles[(h, j)][:],
                        vaug3[:, HPC * i + h, :],
                        nts[(i, h, jt)][:, joff - toff : joff - toff + jw],
                        start=(i == 0),
                        stop=(i == NIK - 1),
                    )
        norm_and_out(grp, av_tiles)
        pav.release()
    ps_pool.release()

    for dname in dbg:
        src = {"qn": qn, "kn": kn, "vn": vn, "avn": avn}[dname]
        eng = gp if dname == "vn" else sy
        eng.dma_start(dram["dbg_" + dname][:, : src.shape[1]], src[:])

    stack.close()


_last_dims = (1024, 1024)


def _build(use_bias, use_affine, debug_names="", Tq=None, Tk=None):
    if Tq is None or Tk is None:
        Tq, Tk = _last_dims
    return _build_impl(use_bias, use_affine, debug_names, Tq, Tk)


@functools.lru_cache(maxsize=4)
def _build_impl(use_bias, use_affine, debug_names, Tq, Tk):
    nc = bass.Bass(
        "TRN2", target_bir_lowering=False, debug=False, num_devices=NCORES
    )
    NIK = Tk // 128
    dram = {}
    dram["xq"] = nc.dram_tensor("xq", [E, Tq], BF, kind="ExternalInput").ap()
    dram["xk"] = nc.dram_tensor("xk", [E, Tk], BF, kind="ExternalInput").ap()
    dram["xv"] = nc.dram_tensor("xv", [E, Tk], BF, kind="ExternalInput").ap()
    dram["wpack"] = nc.dram_tensor(
        "wpack", [128, 4 * E + 128], BF, kind="ExternalInput"
    ).ap()
    dram["cpackr"] = nc.dram_tensor(
        "cpackr", [128, 128 + DH], FR, kind="ExternalInput"
    ).ap()
    dram["mbias"] = nc.dram_tensor("mbias", [128, NIK], FP, kind="ExternalInput").ap()
    if use_bias:
        for bn in ("bcq", "bck", "bcv"):
            dram[bn] = nc.dram_tensor(bn, [128, 1], FP, kind="ExternalInput").ap()
    if use_affine:
        for gn in ("geq", "beq", "gek", "bek", "gev", "bev"):
            dram[gn] = nc.dram_tensor(gn, [128, 1], FP, kind="ExternalInput").ap()
    dram["out"] = nc.dram_tensor("out", [E, Tq], BF, kind="ExternalOutput").ap()
    dbg = frozenset(debug_names.split(",")) - {""} if debug_names else frozenset()
    for dname in dbg:
        w = Tq if dname in ("qn", "avn") else Tk
        dram["dbg_" + dname] = nc.dram_tensor(
            "dbg_" + dname, [128, w], BF, kind="ExternalOutput"
        ).ap()

    flags = {"use_bias": use_bias, "use_affine": use_affine}
    with tile.TileContext(nc) as tc:
        _emit(nc, tc, dram, flags, dbg, Tq, Tk)
    _split_multiwaits(nc)
    return nc


def _pad_up(n, m):
    return max(m, ((n + m - 1) // m) * m)


@functools.lru_cache(maxsize=1)
def _m2_const():
    m2 = np.zeros((128, 128), np.float32)
    m2[:DH, :DH] = 1.0 / DH
    m2[DH:, DH:] = 1.0 / DH
    return m2


def _std(w):
    mu = w.mean(axis=1, keepdims=True)
    var = w.var(axis=1, keepdims=True)
    return (w - mu) / np.sqrt(var + EPS)


_last_results = None


def kernel(**inputs):
    global _last_results, _last_dims
    a = {k: np.asarray(val) for k, val in inputs.items()}
    use_bias = bool(any(np.any(a[bn] != 0) for bn in ("bq", "bk", "bv")))
    use_affine = bool(
        any(np.any(a[gn] != 1) for gn in ("ln_gq", "ln_gk", "ln_gv"))
        or any(np.any(a[bn] != 0) for bn in ("ln_bq", "ln_bk", "ln_bv"))
    )
    debug_names = os.environ.get("KDEBUG", "")

    qm = a["query_mask"].astype(bool)
    km = a["key_mask"].astype(bool)
    kept_q = [np.flatnonzero(qm[b]) for b in range(B)]
    kept_k = [np.flatnonzero(km[b]) for b in range(B)]
    nq = [len(ix) for ix in kept_q]
    nk = [len(ix) for ix in kept_k]
    Tq = _pad_up(max(nq), 128)
    Tk = _pad_up(max(nk), 128)
    NIK = Tk // 128

    _last_dims = (Tq, Tk)
    nc = _build(use_bias, use_affine, debug_names, Tq, Tk)

    m2 = _m2_const()
    i128 = np.eye(128, dtype=np.float32)
    wsn = {wn: _std(a[wn].astype(np.float32)) for wn in ("Wq", "Wk", "Wv", "Wo")}

    in_maps = []
    for c in range(NCORES):
        b, hp = divmod(c, G)
        rs = 128 * hp
        d = {}
        xq = np.zeros((E, Tq), np.float32)
        xq[:, : nq[b]] = a["q"][b][:, kept_q[b]]
        xk = np.zeros((E, Tk), np.float32)
        xk[:, : nk[b]] = a["k"][b][:, kept_k[b]]
        xv = np.zeros((E, Tk), np.float32)
        xv[:, : nk[b]] = (a["v"][b] * a["value_mask"][b][None, :].astype(np.float32))[
            :, kept_k[b]
        ]
        d["xq"], d["xk"], d["xv"] = xq, xk, xv

        wpack = np.empty((128, 4 * E + 128), np.float32)
        for wi, key in enumerate(("Wq", "Wk", "Wv")):
            blk = (i128 - m2) @ wsn[key][rs : rs + 128]  # fold per-head centering
            for k in range(NK):
                wpack[:, wi * E + 128 * k : wi * E + 128 * (k + 1)] = blk[
                    :, 128 * k : 128 * (k + 1)
                ].T
        for m in range(4):
            wpack[:, 3 * E + 128 * m : 3 * E + 128 * (m + 1)] = wsn["Wo"][
                128 * m : 128 * (m + 1), rs : rs + 128
            ].T
        wpack[:, 4 * E : 4 * E + 128] = m2
        d["wpack"] = wpack

        cpackr = np.zeros((128, 128 + DH), np.float32)
        cpackr[:, 0:128] = i128
        cpackr[:, 128:] = 1.0
        d["cpackr"] = cpackr
        flat = np.zeros(Tk, np.float32)
        flat[nk[b] :] = -BIG
        d["mbias"] = flat.reshape(NIK, 128).T

        if use_bias:
            for bn, key in (("bcq", "bq"), ("bck", "bk"), ("bcv", "bv")):
                bb = a[key].astype(np.float32)[rs : rs + 128]
                d[bn] = (bb - m2 @ bb)[:, None]
        if use_affine:
            # q's cscale stays 1/SCALE; out_q = (LN/SCALE)*g + b/SCALE
            d["geq"] = np.tile(a["ln_gq"], HPC)[:, None]
            d["beq"] = (np.tile(a["ln_bq"], HPC) / SCALE)[:, None]
            d["gek"] = np.tile(a["ln_gk"], HPC)[:, None]
            d["bek"] = np.tile(a["ln_bk"], HPC)[:, None]
            d["gev"] = np.tile(a["ln_gv"], HPC)[:, None]
            d["bev"] = np.tile(a["ln_bv"], HPC)[:, None]
        for k in ("xq", "xk", "xv", "wpack"):
            d[k] = np.ascontiguousarray(d[k]).astype(BF_NP)
        for k in ("cpackr", "mbias", "bcq", "bck", "bcv",
                  "geq", "beq", "gek", "bek", "gev", "bev"):
            if k in d:
                d[k] = np.ascontiguousarray(d[k], dtype=np.float32)
        in_maps.append(d)

    res = bass_utils.run_bass_kernel_spmd(
        nc,
        in_maps,
        core_ids=list(range(NCORES)),
        trace=os.environ.get("KTRACE", "0") == "1",
    )
    _last_results = res
    kernel._last_meta = {"Tq": Tq, "Tk": Tk, "nq": nq, "nk": nk,
                         "kept_q": kept_q, "kept_k": kept_k}

    out = np.zeros((B, E, T), np.float32)
    bo = a["bo"].astype(np.float32)
    for b in range(B):
        acc = res.results[G * b]["out"].astype(np.float32)
        for c in range(G * b + 1, G * b + G):
            acc = acc + res.results[c]["out"].astype(np.float32)
        out[b][:, kept_q[b]] = acc[:, : nq[b]]
        out[b] += bo[:, None]
    return out


# revision 81
# speedup vs baseline: 2.1632x; 1.0017x over previous
"""nn_MultiHeadAttention_84954453115654 — Trainium2 Bass kernel, 8 NeuronCores.

Sharding: data-parallel over batch (2) x head-pair-parallel (4 groups of 2
heads).  Core c handles batch b = c//4 and embed rows [128*(c%4), +128)
(= heads 2*(c%4), 2*(c%4)+1).  Host sums the 4 out_proj partials per batch
and adds bo (the row-parallel all-reduce, done at gather time).

Key idea: the masks are inputs, so the host COMPACTS the time axis before
launch.  Only unmasked query columns (Tq ~ half of 2048) and unmasked key
columns (Tk ~ half) are shipped/computed; value_mask is pre-applied to the
compacted xv.  Masked query columns of the output are exactly bo (reference:
softmax scores row zeroed -> av col 0 -> out col = bias), so the host
scatters computed columns back and fills the rest with bo.  This halves DMA
and PE work and quarters the softmax-exp ACT work vs. dense.

Weight prep happens on host (weights are inputs): weight-standardize, fold
the per-head centering (I - M2) into Wq/Wk/Wv so the projection matmul
directly yields diff = p - mean_head(p), transpose into lhsT layout, bf16.

Per-core device program:
  - q/k/v projections (bf16 matmuls, K-chained over 4 tiles of 128) with
    fused per-head LayerNorm: var via block-mean matmul of diff^2, then
    out = (diff * cscale) * reciprocal(sqrt(var + eps)) (1/SCALE folded
    into q's cscale)
  - scores transposed per (tk-tile i, head, tq-tile): S^T[tk, tq] =
    kn_i^T @ qn chunk; softmax exp on ACT with a per-partition bias (-80
    on compact-pad keys, else 0); no row-max subtraction (post-LN scores
    are O(0.1) so exp cannot overflow)
  - av[65, jw] += vaugT_i @ exp accumulated over i in PSUM; vaug carries a
    ones column so row 64 is the softmax denominator; av matmuls are
    spread between exp units to keep the ACT stream dense
  - normalize: denominator rows batched into one tile, one reciprocal,
    PE-broadcast per row, avn = avsb * bcast
  - out_proj partial: woT[:, m] @ avn -> PSUM, DMA'd straight to DRAM
    (bf16 cast in the DMA, via gpsimd)
"""

import os
import sys
import contextlib
import functools

for _p in ("/root/.axon_site/_ro/trn_rl_repo", "/opt/trn_rl_repo"):
    if os.path.isdir(_p) and _p not in sys.path:
        sys.path.append(_p)

import numpy as np
import ml_dtypes

import concourse.bass as bass
import concourse.mybir as mybir
import concourse.tile as tile
from concourse import bass_utils

B, E, T, H = 2, 512, 2048, 8
DH = E // H            # 64
HPC = 2                # heads per core
G = 4                  # cores per batch group
NCORES = 8
NK = E // 128          # 4 contraction tiles for the projections
EPS = 1e-5
SCALE = float(E // H ** 0.5)   # 181.0
BIG = 80.0
FP = mybir.dt.float32
FR = mybir.dt.float32r
BF = mybir.dt.bfloat16
AF = mybir.ActivationFunctionType
OP = mybir.AluOpType
BF_NP = np.dtype(ml_dtypes.bfloat16)


def _split_multiwaits(nc):
    """Split multi-wait instructions (Tile's tail drain) into single-wait
    EventSemaphore chains; this container's walrus encodes only one sync
    wait per instruction."""
    import bass_rust

    n_new = 0
    for f in nc.m.functions:
        for bb in f.blocks:
            out = []
            changed = False
            for ins in bb.instructions:
                si = ins.sync_info
                if si is not None and si.on_wait is not None and len(si.on_wait) > 1:
                    waits = list(si.on_wait)
                    for w in waits[:-1]:
                        ev = bass_rust.InstEventSemaphore(
                            name=f"MWFIX-{n_new}", ins=[], outs=[]
                        )
                        n_new += 1
                        ev.engine = ins.engine
                        ev.sync_info = bass_rust.SyncInfo(on_wait=[w], on_update=[])
                        out.append(ev)
                    ins.sync_info = bass_rust.SyncInfo(
                        on_wait=[waits[-1]], on_update=list(si.on_update or [])
                    )
                    changed = True
                out.append(ins)
            if changed:
                bb.instructions = out
    return n_new


def _chunks(total, step=512):
    out = []
    off = 0
    while off < total:
        out.append((off, min(step, total - off)))
        off += step
    return out


def _jt_split(Tq):
    """scores/exp tile widths: one tile if it fits 2 PSUM banks, else
    (512, rest) so the first exp only needs the first qn chunk."""
    if Tq <= 1024:
        return [(0, Tq)]
    assert Tq <= 1536, f"Tq={Tq} too large for the 2-tile scores layout"
    return [(0, 512), (512, Tq - 512)]


def _emit(nc, tc, dram, flags, dbg, Tq, Tk):
    v = nc.vector
    sc = nc.scalar
    te = nc.tensor
    gp = nc.gpsimd
    sy = nc.sync

    NIK = Tk // 128          # tk tiles
    JQ = _chunks(Tq)         # tq chunks (512-wide, last may be ragged)
    JK = _chunks(Tk)
    JT = _jt_split(Tq)
    use_bias = flags["use_bias"]
    use_affine = flags["use_affine"]

    stack = contextlib.ExitStack()

    consts = stack.enter_context(tc.tile_pool(name="consts", bufs=1))
    xcp = stack.enter_context(tc.tile_pool(name="xc", bufs=2))
    nbuf = stack.enter_context(tc.tile_pool(name="named", bufs=1))
    ntp = stack.enter_context(
        tc.tile_pool(name="nt", bufs=(Tk // 128) * HPC * len(_jt_split(Tq)) + 2)
    )
    sdp = stack.enter_context(tc.tile_pool(name="sd", bufs=3))
    dnp = stack.enter_context(tc.tile_pool(name="dn", bufs=1))

    # ---- packed constant loads -----------------------------------------
    # wqk: [wq | wk | m2b] bf16 (needed before the first exp);
    # wvo: [wv | woT] bf16 (queued after the early x chunks);
    # cpackr: [identr | onesP] fp32r; mbias separate fp32.
    cpackr = consts.tile([128, 128 + DH], FR, tag="cpackr")
    sy.dma_start(cpackr[:], dram["cpackr"])
    identr = cpackr[:, 0:128]
    onesP = cpackr[:, 128 : 128 + DH]  # all-ones rows (base-aligned lhsT)
    mbias = consts.tile([128, NIK], FP, tag="mbias")
    sy.dma_start(mbias[:], dram["mbias"])
    wqk = consts.tile([128, 2 * E + 128], BF, tag="wqk")
    sy.dma_start(wqk[:], dram["wqk"])
    wvo = consts.tile([128, 2 * E], BF, tag="wvo")
    wT = {"wq": wqk[:, 0:E], "wk": wqk[:, E : 2 * E], "wv": wvo[:, 0:E]}
    m2b = wqk[:, 2 * E : 2 * E + 128]
    woT = wvo[:, E : 2 * E]
    epst = consts.tile([128, 1], FP, tag="eps")
    v.memset(epst[:], EPS)

    bias_tiles = {}
    if use_bias:
        for bn in ("bcq", "bck", "bcv"):
            bt = consts.tile([128, 1], FP, tag=bn, name=bn)
            sy.dma_start(bt[:], dram[bn])
            bias_tiles[bn] = bt
    ge_tiles = {}
    if use_affine:
        for gn in ("geq", "beq", "gek", "bek", "gev", "bev"):
            gt = consts.tile([128, 1], FP, tag=gn, name=gn)
            sy.dma_start(gt[:], dram[gn])
            ge_tiles[gn] = gt

    # ---- x loads -------------------------------------------------------
    # Host packs x as [128, nchunks, NK, 512] so ONE DMA delivers a whole
    # projection chunk (all 4 k-tiles).  Order: xq c0, xk c0, xq c1.., then
    # xk c1.., then wvo + xv chunks (v is consumed last).
    xc_tiles = {}

    def xload(tn, j):
        xt = xcp.tile([128, NK, 512], BF, tag=f"xc_{tn}", name=f"x_{tn}_{j}")
        sy.dma_start(xt[:], dram[tn][:, j, :, :])
        xc_tiles[(tn, j)] = xt

    xload("xq", 0)
    xload("xk", 0)
    for j in range(1, len(JQ)):
        xload("xq", j)
    for j in range(1, len(JK)):
        xload("xk", j)
    sy.dma_start(wvo[:], dram["wvo"])
    for j in range(len(JK)):
        xload("xv", j)

    # named projection outputs
    qn = nbuf.tile([128, Tq], BF, tag="qn", name="qn")
    kn = nbuf.tile([128, Tk], BF, tag="kn", name="kn")
    vn = nbuf.tile([128, Tk], FR, tag="vn", name="vn")  # fp32r: feeds transpose
    # vaugT: per (i, h) a [128, 65] block: cols 0..63 = v^T, col 64 = ones
    vaug = nbuf.tile([128, NIK * HPC * 65], BF, tag="vaug", name="vaug")
    vaug3 = vaug[:].rearrange("p (n c) -> p n c", c=65)
    gp.memset(vaug[:], 1.0)
    avn = nbuf.tile([128, Tq], BF, tag="avn", name="avn")

    # PSUM pool stack: ps at the bottom (also hosts rbp/po at the tail),
    # pj on top (released once all projections are emitted), then pav.
    ps_pool = tc.alloc_tile_pool(name="ps", bufs=2, space="PSUM")
    pj = tc.alloc_tile_pool(name="pj", bufs=2, space="PSUM")

    def project_chunk(tn, wname, joff, jw, cscale, bn, gn, bln, dsq_eng):
        """Project x cols [joff, joff+jw) and apply per-head LN."""
        out = {"xq": qn, "xk": kn, "xv": vn}[tn]
        j = joff // 512
        pp = pj.tile([128, 512], FP, tag="pp", name=f"pp_{tn}_{j}")
        for k in range(NK):
            te.matmul(
                pp[:, :jw],
                wT[wname][:, 128 * k : 128 * (k + 1)],
                xc_tiles[(tn, j)][:, k, :jw],
                start=(k == 0),
                stop=(k == NK - 1),
            )
        if use_bias or dsq_eng is not sc:
            # SBUF copy of diff: two-PSUM-input ops are illegal, so non-ACT
            # squares (and the bias add) go through SBUF
            u = sdp.tile([128, 512], FP, tag="u", name=f"u_{tn}_{j}")
            v.tensor_scalar_add(
                u[:, :jw], pp[:, :jw], bias_tiles[bn][:] if use_bias else 0.0
            )
            diff = u
        else:
            diff = pp
        dsq = sdp.tile([128, 512], BF, tag="dsq", name=f"dsq_{tn}_{j}")
        if dsq_eng is sc:
            sc.activation(dsq[:, :jw], diff[:, :jw], AF.Square)
        else:
            dsq_eng.tensor_mul(dsq[:, :jw], diff[:, :jw], diff[:, :jw])
        pvar = pj.tile([128, 512], FP, tag="pvar", bufs=1, name=f"pvar_{tn}_{j}")
        te.matmul(pvar[:, :jw], m2b, dsq[:, :jw], start=True, stop=True)
        sd = sdp.tile([128, 512], FP, tag="sd", name=f"sd_{tn}_{j}")
        sc.activation(sd[:, :jw], pvar[:, :jw], AF.Sqrt, bias=epst[:])
        rsq = sdp.tile([128, 512], FP, tag="rsq", name=f"rsq_{tn}_{j}")
        v.reciprocal(rsq[:, :jw], sd[:, :jw])
        v.scalar_tensor_tensor(
            out[:, joff : joff + jw],
            diff[:, :jw],
            cscale,
            rsq[:, :jw],
            op0=OP.mult,
            op1=OP.mult,
        )
        if use_affine:
            v.tensor_scalar(
                out[:, joff : joff + jw],
                out[:, joff : joff + jw],
                ge_tiles[gn][:],
                ge_tiles[bln][:],
                op0=OP.mult,
                op1=OP.add,
            )

    def vchunk(joff, jw):
        """v projection chunk + transposes of its tk tiles into vaug."""
        project_chunk("xv", "wv", joff, jw, 1.0, "bcv", "gev", "bev", gp)
        for ii in range(jw // 128):
            i = joff // 128 + ii
            pt = pj.tile([128, 128], FR, tag="ptr", bufs=1, name=f"ptr{i}")
            te.transpose(pt[:], vn[:, 128 * i : 128 * (i + 1)], identr)
            src = pt[:].bitcast(FP).rearrange("p (h c) -> p h c", c=DH)
            dst = vaug3[:, HPC * i : HPC * i + HPC, 0:DH]
            v.tensor_copy(dst, src)

    # ---- emission: q c0 + k c0 only; remaining q chunks land after the
    # warm-start score units (which need only qn cols 0..512)
    project_chunk("xq", "wq", *JQ[0], 1.0 / SCALE, "bcq", "geq", "beq", sc)
    project_chunk("xk", "wk", *JK[0], 1.0, "bck", "gek", "bek", sc)

    # ---- attention -----------------------------------------------------
    groups = [list(enumerate(JQ))[g0 : g0 + 2] for g0 in range(0, len(JQ), 2)]
    grp0 = groups[0]

    nts = {}
    av_pending = []
    av_tiles = {}

    def av_push(i):
        for h in range(HPC):
            for j, (joff, jw) in grp0:
                av_pending.append((i, h, j, joff, jw))

    def av_emit(limit):
        n = 0
        while av_pending and n < limit:
            i, h, j, joff, jw = av_pending.pop(0)
            jt = 0 if joff < JT[0][1] else 1
            toff = JT[jt][0]
            te.matmul(
                av_tiles[(h, j)][:],
                vaug3[:, HPC * i + h, :],
                nts[(i, h, jt)][:, joff - toff : joff - toff + jw],
                start=(i == 0),
                stop=(i == NIK - 1),
            )
            n += 1

    def unit(i, h, jt):
        hs = slice(DH * h, DH * (h + 1))
        toff, tw = JT[jt]
        ps = ps_pool.tile([128, 1024], FP, tag="ps", name=f"ps{i}_{h}_{jt}")
        for soff, sw in _chunks(tw):
            te.matmul(
                ps[:, soff : soff + sw],
                kn[hs, 128 * i : 128 * (i + 1)],
                qn[hs, toff + soff : toff + soff + sw],
                start=True,
                stop=True,
            )
        nt = ntp.tile([128, 1024], BF, tag="nt", name=f"nt{i}_{h}_{jt}")
        sc.activation(nt[:, :tw], ps[:, :tw], AF.Exp, bias=mbias[:, i : i + 1])
        nts[(i, h, jt)] = nt
        if pav is not None:
            av_emit(3)

    # warm start: units needing only qn c0 / kn c0, while qn c1+ projects
    pav = None
    warm = min(2, NIK)
    for i in range(warm):
        for h in range(HPC):
            unit(i, h, 0)
    for joff, jw in JQ[1:]:
        project_chunk("xq", "wq", joff, jw, 1.0 / SCALE, "bcq", "geq", "beq", sc)
    for i in range(warm):
        for h in range(HPC):
            for jt in range(1, len(JT)):
                unit(i, h, jt)
        av_push(i)

    vi = 0
    ki = 1
    for i in range(warm, NIK):
        # interleave later k/v projection chunks; spacing keeps the PE wait
        # on their DMAs behind scores the ACT stream has already consumed
        if ki < len(JK) and i == ki + 2:
            project_chunk("xk", "wk", *JK[ki], 1.0, "bck", "gek", "bek", v)
            ki += 1
        if i >= 2 and vi < len(JK):
            vchunk(*JK[vi])
            vi += 1
        for h in range(HPC):
            for jt in range(len(JT)):
                unit(i, h, jt)
        av_push(i)
        if pav is None and vi == len(JK) and ki == len(JK):
            pj.release()
            pav = tc.alloc_tile_pool(name="pav0", bufs=1, space="PSUM")
            for h in range(HPC):
                for j, (joff, jw) in grp0:
                    av_tiles[(h, j)] = pav.tile(
                        [DH + 1, jw], FP, tag=f"av{h}_{j}", name=f"av{h}_{j}"
                    )
    if pav is None:
        while ki < len(JK):
            project_chunk("xk", "wk", *JK[ki], 1.0, "bck", "gek", "bek", v)
            ki += 1
        while vi < len(JK):
            vchunk(*JK[vi])
            vi += 1
        pj.release()
        pav = tc.alloc_tile_pool(name="pav0", bufs=1, space="PSUM")
        for h in range(HPC):
            for j, (joff, jw) in grp0:
                av_tiles[(h, j)] = pav.tile(
                    [DH + 1, jw], FP, tag=f"av{h}_{j}", name=f"av{h}_{j}"
                )
    av_emit(10**9)

    # ---- normalize + out_proj + store ---------------------------------
    # Denominator rows are gathered into one tile per group (row r), one
    # reciprocal, then a per-row PE broadcast (onesP rows keep base
    # partitions aligned).  out_proj PSUM rides the ps tag ring and is
    # DMA'd straight to DRAM with a bf16 cast (gpsimd-initiated).
    outp = stack.enter_context(tc.tile_pool(name="outsb", bufs=2))

    def norm_emit(grp, av_tiles):
        pairs = [(h, j, joff, jw) for j, (joff, jw) in grp for h in range(HPC)]
        # recip rows live at (partition 32*(r%2), col slot 512*(r//2)) so the
        # broadcast matmul operands sit at legal base partitions (0/32)
        rw = 512 * ((len(pairs) + 1) // 2)
        rcp = dnp.tile([128, 1024], FR, tag="rcp", name="rcp")
        rslc = lambda r, jw: rcp[
            32 * (r % 2) : 32 * (r % 2) + 1, 512 * (r // 2) : 512 * (r // 2) + jw
        ]
        avsbs = {}
        for r, (h, j, joff, jw) in enumerate(pairs):
            avt = av_tiles[(h, j)]
            avsb = dnp.tile([DH, 512], FP, tag=f"avsb{r % 4}", name=f"avsb{h}_{j}")
            avsbs[(h, j)] = avsb
            if r % 2 == 0:
                v.tensor_copy(avsb[:, :jw], avt[0:DH, :])
                sc.copy(rslc(r, jw), avt[DH : DH + 1, :])
            else:
                sc.copy(avsb[:, :jw], avt[0:DH, :])
                v.tensor_copy(rslc(r, jw), avt[DH : DH + 1, :])
        with nc.allow_low_precision(reason="fp32r recip rows for PE broadcast"):
            v.reciprocal(rcp[0:33, :rw], rcp[0:33, :rw])
        for r, (h, j, joff, jw) in enumerate(pairs):
            hs = slice(DH * h, DH * (h + 1))
            rbp = ps_pool.tile([128, 1024], FP, tag="ps", name=f"rbp{h}_{j}")
            te.matmul(
                rbp[0:DH, :jw],
                onesP[32 * (r % 2) : 32 * (r % 2) + 1, :],
                rslc(r, jw),
                start=True,
                stop=True,
            )
            v.tensor_mul(
                avn[hs, joff : joff + jw], avsbs[(h, j)][:, :jw], rbp[0:DH, :jw]
            )

    def out_emit(grp):
        # out DMA merged per j-chunk: drains write [128, 4, 512] slices
        for j, (joff, jw) in grp:
            ot = outp.tile([128, 4, 512], BF, tag="ot", name=f"ot{j}")
            for m in range(4):
                po = ps_pool.tile([128, 1024], FP, tag="ps", name=f"po{j}_{m}")
                te.matmul(
                    po[:, :jw],
                    woT[:, 128 * m : 128 * (m + 1)],
                    avn[:, joff : joff + jw],
                    start=True,
                    stop=True,
                )
                if m % 2 == 0:
                    v.tensor_copy(ot[:, m, :jw], po[:, :jw])
                else:
                    sc.copy(ot[:, m, :jw], po[:, :jw])
            sy.dma_start(dram["out"][:, j, :, :], ot[:])

    # grp1 (if any) is emitted between grp0's normalize and out_proj so its
    # av chains and normalize overlap grp0's out stage
    norm_emit(grp0, av_tiles)
    pav.release()
    if len(groups) > 1:
        grp = groups[1]
        pav1 = tc.alloc_tile_pool(name="pav1", bufs=1, space="PSUM")
        av1 = {
            (h, j): pav1.tile([DH + 1, jw], FP, tag=f"av{h}_{j}", name=f"avx{h}_{j}")
            for j, (joff, jw) in grp
            for h in range(HPC)
        }
        for i in range(NIK):
            for h in range(HPC):
                for j, (joff, jw) in grp:
                    jt = 0 if joff < JT[0][1] else 1
                    toff = JT[jt][0]
                    te.matmul(
                        av1[(h, j)][:],
                        vaug3[:, HPC * i + h, :],
                        nts[(i, h, jt)][:, joff - toff : joff - toff + jw],
                        start=(i == 0),
                        stop=(i == NIK - 1),
                    )
        out_emit(grp0)
        norm_emit(grp, av1)
        pav1.release()
        out_emit(grp)
    else:
        out_emit(grp0)
    ps_pool.release()

    for dname in dbg:
        src = {"qn": qn, "kn": kn, "vn": vn, "avn": avn}[dname]
        eng = gp if dname == "vn" else sy
        eng.dma_start(dram["dbg_" + dname][:, : src.shape[1]], src[:])

    stack.close()


_last_dims = (1024, 1024)


def _build(use_bias, use_affine, debug_names="", Tq=None, Tk=None):
    if Tq is None or Tk is None:
        Tq, Tk = _last_dims
    return _build_impl(use_bias, use_affine, debug_names, Tq, Tk)


@functools.lru_cache(maxsize=4)
def _build_impl(use_bias, use_affine, debug_names, Tq, Tk):
    nc = bass.Bass(
        "TRN2", target_bir_lowering=False, debug=False, num_devices=NCORES
    )
    NIK = Tk // 128
    NJQ = (Tq + 511) // 512
    NJK = (Tk + 511) // 512
    dram = {}
    dram["xq"] = nc.dram_tensor("xq", [128, NJQ, NK, 512], BF, kind="ExternalInput").ap()
    dram["xk"] = nc.dram_tensor("xk", [128, NJK, NK, 512], BF, kind="ExternalInput").ap()
    dram["xv"] = nc.dram_tensor("xv", [128, NJK, NK, 512], BF, kind="ExternalInput").ap()
    dram["wqk"] = nc.dram_tensor(
        "wqk", [128, 2 * E + 128], BF, kind="ExternalInput"
    ).ap()
    dram["wvo"] = nc.dram_tensor("wvo", [128, 2 * E], BF, kind="ExternalInput").ap()
    dram["cpackr"] = nc.dram_tensor(
        "cpackr", [128, 128 + DH], FR, kind="ExternalInput"
    ).ap()
    dram["mbias"] = nc.dram_tensor("mbias", [128, NIK], FP, kind="ExternalInput").ap()
    if use_bias:
        for bn in ("bcq", "bck", "bcv"):
            dram[bn] = nc.dram_tensor(bn, [128, 1], FP, kind="ExternalInput").ap()
    if use_affine:
        for gn in ("geq", "beq", "gek", "bek", "gev", "bev"):
            dram[gn] = nc.dram_tensor(gn, [128, 1], FP, kind="ExternalInput").ap()
    dram["out"] = nc.dram_tensor(
        "out", [128, NJQ, 4, 512], BF, kind="ExternalOutput"
    ).ap()
    dbg = frozenset(debug_names.split(",")) - {""} if debug_names else frozenset()
    for dname in dbg:
        w = Tq if dname in ("qn", "avn") else Tk
        dram["dbg_" + dname] = nc.dram_tensor(
            "dbg_" + dname, [128, w], BF, kind="ExternalOutput"
        ).ap()

    flags = {"use_bias": use_bias, "use_affine": use_affine}
    with tile.TileContext(nc) as tc:
        _emit(nc, tc, dram, flags, dbg, Tq, Tk)
    _split_multiwaits(nc)
    return nc


def _pad_up(n, m):
    return max(m, ((n + m - 1) // m) * m)


@functools.lru_cache(maxsize=1)
def _m2_const():
    m2 = np.zeros((128, 128), np.float32)
    m2[:DH, :DH] = 1.0 / DH
    m2[DH:, DH:] = 1.0 / DH
    return m2


def _std(w):
    mu = w.mean(axis=1, keepdims=True)
    var = w.var(axis=1, keepdims=True)
    return (w - mu) / np.sqrt(var + EPS)


_last_results = None


def kernel(**inputs):
    global _last_results, _last_dims
    a = {k: np.asarray(val) for k, val in inputs.items()}
    use_bias = bool(any(np.any(a[bn] != 0) for bn in ("bq", "bk", "bv")))
    use_affine = bool(
        any(np.any(a[gn] != 1) for gn in ("ln_gq", "ln_gk", "ln_gv"))
        or any(np.any(a[bn] != 0) for bn in ("ln_bq", "ln_bk", "ln_bv"))
    )
    debug_names = os.environ.get("KDEBUG", "")

    qm = a["query_mask"].astype(bool)
    km = a["key_mask"].astype(bool)
    kept_q = [np.flatnonzero(qm[b]) for b in range(B)]
    kept_k = [np.flatnonzero(km[b]) for b in range(B)]
    nq = [len(ix) for ix in kept_q]
    nk = [len(ix) for ix in kept_k]
    Tq = _pad_up(max(nq), 128)
    Tk = _pad_up(max(nk), 128)
    NIK = Tk // 128

    _last_dims = (Tq, Tk)
    nc = _build(use_bias, use_affine, debug_names, Tq, Tk)

    m2 = _m2_const()
    i128 = np.eye(128, dtype=np.float32)
    wsn = {wn: _std(a[wn].astype(np.float32)) for wn in ("Wq", "Wk", "Wv", "Wo")}

    NJQ = (Tq + 511) // 512
    NJK = (Tk + 511) // 512

    def xpack(x, n, NJ):
        # [E, n<=512*NJ] -> [128, NJ, NK, 512] with zero padding
        full = np.zeros((E, 512 * NJ), np.float32)
        full[:, :n] = x[:, :n]
        return np.ascontiguousarray(
            full.reshape(NK, 128, NJ, 512).transpose(1, 2, 0, 3)
        )

    in_maps = []
    for c in range(NCORES):
        b, hp = divmod(c, G)
        rs = 128 * hp
        d = {}
        d["xq"] = xpack(a["q"][b][:, kept_q[b]], nq[b], NJQ)
        d["xk"] = xpack(a["k"][b][:, kept_k[b]], nk[b], NJK)
        d["xv"] = xpack(
            (a["v"][b] * a["value_mask"][b][None, :].astype(np.float32))[:, kept_k[b]],
            nk[b],
            NJK,
        )

        wqk = np.empty((128, 2 * E + 128), np.float32)
        wvo = np.empty((128, 2 * E), np.float32)
        for wi, key in enumerate(("Wq", "Wk", "Wv")):
            blk = (i128 - m2) @ wsn[key][rs : rs + 128]  # fold per-head centering
            dst = wqk if wi < 2 else wvo
            base = (wi % 2) * E if wi < 2 else 0
            for k in range(NK):
                dst[:, base + 128 * k : base + 128 * (k + 1)] = blk[
                    :, 128 * k : 128 * (k + 1)
                ].T
        for m in range(4):
            wvo[:, E + 128 * m : E + 128 * (m + 1)] = wsn["Wo"][
                128 * m : 128 * (m + 1), rs : rs + 128
            ].T
        wqk[:, 2 * E : 2 * E + 128] = m2
        d["wqk"] = wqk
        d["wvo"] = wvo

        cpackr = np.zeros((128, 128 + DH), np.float32)
        cpackr[:, 0:128] = i128
        cpackr[:, 128:] = 1.0
        d["cpackr"] = cpackr
        flat = np.zeros(Tk, np.float32)
        flat[nk[b] :] = -BIG
        d["mbias"] = flat.reshape(NIK, 128).T

        if use_bias:
            for bn, key in (("bcq", "bq"), ("bck", "bk"), ("bcv", "bv")):
                bb = a[key].astype(np.float32)[rs : rs + 128]
                d[bn] = (bb - m2 @ bb)[:, None]
        if use_affine:
            # q's cscale stays 1/SCALE; out_q = (LN/SCALE)*g + b/SCALE
            d["geq"] = np.tile(a["ln_gq"], HPC)[:, None]
            d["beq"] = (np.tile(a["ln_bq"], HPC) / SCALE)[:, None]
            d["gek"] = np.tile(a["ln_gk"], HPC)[:, None]
            d["bek"] = np.tile(a["ln_bk"], HPC)[:, None]
            d["gev"] = np.tile(a["ln_gv"], HPC)[:, None]
            d["bev"] = np.tile(a["ln_bv"], HPC)[:, None]
        for k in ("xq", "xk", "xv", "wqk", "wvo"):
            d[k] = np.ascontiguousarray(d[k]).astype(BF_NP)
        for k in ("cpackr", "mbias", "bcq", "bck", "bcv",
                  "geq", "beq", "gek", "bek", "gev", "bev"):
            if k in d:
                d[k] = np.ascontiguousarray(d[k], dtype=np.float32)
        in_maps.append(d)

    res = bass_utils.run_bass_kernel_spmd(
        nc,
        in_maps,
        core_ids=list(range(NCORES)),
        trace=os.environ.get("KTRACE", "0") == "1",
    )
    _last_results = res
    kernel._last_meta = {"Tq": Tq, "Tk": Tk, "nq": nq, "nk": nk,
                         "kept_q": kept_q, "kept_k": kept_k}

    out = np.zeros((B, E, T), np.float32)
    bo = a["bo"].astype(np.float32)
    for b in range(B):
        acc = res.results[G * b]["out"].astype(np.float32)
        for c in range(G * b + 1, G * b + G):
            acc = acc + res.results[c]["out"].astype(np.float32)
        # [128, NJQ, 4, 512] -> [E, 512*NJQ]
        full = acc.transpose(2, 0, 1, 3).reshape(E, 512 * NJQ)
        out[b][:, kept_q[b]] = full[:, : nq[b]]
        out[b] += bo[:, None]
    return out
